# revision 1
# baseline (speedup 1.0000x reference)
"""Causal multi-head attention with RoPE for Trainium2, 8-core SPMD.

Problem: B=2, S=2048, D_MODEL=1024, H=16, HD=64, causal softmax(QK^T/8)V
with interleaved-pair RoPE on q/k, projections Wq/Wk/Wv/Wo.

Sharding (host side): batch x head-group. Core c handles batch b=c//4 and
head group g=c%4 (heads 4g..4g+3, a 256-wide slice of the projection dims).
Each core computes a full [S, D_MODEL] partial of the output (its head
group's contribution through Wo); host sums 4 partials per batch.

Device layout strategy (all matmuls bf16, fp32 accumulate):
 - host passes x[b].T so the d-contraction sits on SBUF partitions
 - Q,K projected in [s, o] layout -> RoPE on DVE along free dim (pairs are
   adjacent columns) -> bf16 -> DMA-transposed (XBAR, bf16) into [o, s]
 - scores^T[k, q] = Kt.T @ Qt per 128-key block (K=64 contraction); the
   two heads of a pair are issued to PE row groups 0/64 (tile_position)
   and run concurrently. Blocks land in wide PSUM tiles, one Exp per wide
   tile (ACT amortizes its 352-cycle fixed cost), causal-masked by
   multiplying the diagonal 128x128 block; q-columns below the diagonal
   are never computed or consumed
 - PV: lhsT = [V | 1] per key block (M=65) so row 64 of the PSUM output
   accumulates the softmax denominator for free; DVE normalizes
 - o_proj consumes the attention output, PSUM is DMA'd straight to DRAM
"""

import numpy as np
import ml_dtypes

B, S, D, H = 2, 2048, 1024, 16
HD = 64
NCORES = 8
HEADS_PER_CORE = 4
GDIM = HEADS_PER_CORE * HD          # 256 projection cols per core
SB = S // 128                        # 16 s-tiles
KD = D // 128                        # 8 k-tiles over d
QCHUNK = 512
NQC = S // QCHUNK                    # 4 q-chunks
WIDE = 1024                          # wide scores psum tile (2 banks)

_BF16 = ml_dtypes.bfloat16
_cache = {}


def _build(use_rope: bool, reps: int = 1, timing: bool = False, phases=(1, 2, 3)):
    import concourse.bass as bass
    import concourse.mybir as mybir
    import concourse.tile as tile
    from concourse import bacc

    F32 = mybir.dt.float32
    BF16 = mybir.dt.bfloat16
    EXP = mybir.ActivationFunctionType.Exp

    nc = bacc.Bacc(None, target_bir_lowering=False)

    xt_d = nc.dram_tensor("xt", [D, S], BF16, kind="ExternalInput")
    wqk_d = nc.dram_tensor("wqk", [D, 2 * GDIM], BF16, kind="ExternalInput")
    wv_d = nc.dram_tensor("wv", [D, GDIM], BF16, kind="ExternalInput")
    wo_d = nc.dram_tensor("wo", [GDIM, D], BF16, kind="ExternalInput")
    cos_d = nc.dram_tensor("cos8", [S, 256], BF16, kind="ExternalInput")
    sin_d = nc.dram_tensor("sin8", [S, 256], BF16, kind="ExternalInput")
    mask_d = nc.dram_tensor("maskT", [128, 128], BF16, kind="ExternalInput")
    if timing:
        # timing builds: full-size output stays on device (internal DRAM);
        # tiny external output avoids 64MB host transfers per timed call
        out_d = nc.dram_tensor("oscratch", [S, D], F32)
        out_small = nc.dram_tensor("out", [128, 512], F32, kind="ExternalOutput")
    else:
        out_d = nc.dram_tensor("out", [S, D], F32, kind="ExternalOutput")
        out_small = None

    with tile.TileContext(nc) as tc:
        with tc.tile_pool(name="big", bufs=1) as big, \
             tc.tile_pool(name="work", bufs=3) as work, \
             tc.tile_pool(name="ropet", bufs=4) as ropet, \
             tc.tile_pool(name="pex", bufs=4) as pex:
            # ---- resident tensors ----
            xt = big.tile([128, KD, S], BF16)
            nc.sync.dma_start(xt[:], xt_d.rearrange("(k p) s -> p k s", p=128))
            wqk = big.tile([128, KD, 2 * GDIM], BF16)
            nc.sync.dma_start(wqk[:], wqk_d.rearrange("(k p) o -> p k o", p=128))
            wv = big.tile([128, KD, GDIM], BF16)
            nc.sync.dma_start(wv[:], wv_d.rearrange("(k p) o -> p k o", p=128))
            wo = big.tile([128, 2, D], BF16)
            nc.sync.dma_start(wo[:], wo_d.rearrange("(k p) o -> p k o", p=128))
            maskT = big.tile([128, 128], BF16)
            nc.sync.dma_start(maskT[:], mask_d[:])
            if use_rope:
                cos8 = big.tile([128, SB, 256], BF16)
                nc.sync.dma_start(cos8[:], cos_d.rearrange("(m p) f -> p m f", p=128))
                sin8 = big.tile([128, SB, 256], BF16)
                nc.sync.dma_start(sin8[:], sin_d.rearrange("(m p) f -> p m f", p=128))

            # attention-side resident tiles
            qkt = [big.tile([128, S], BF16, tag=f"qkt{i}", name=f"qkt{i}")
                   for i in range(4)]
            # qkt[0]: Qt heads 0-1, qkt[1]: Qt heads 2-3, qkt[2]: Kt 0-1, qkt[3]: Kt 2-3
            vsb = big.tile([128, SB, HEADS_PER_CORE * 65], BF16)
            yt2 = [big.tile([128, S], BF16, tag=f"yt2{i}", name=f"yt2{i}")
                   for i in range(2)]

            for _rep in range(reps):
                # PSUM plan: sc(4 banks)+yt(2)+pp(2) live together; pp closes
                # after phase 1 and op(2) reuses its banks, so projections,
                # attention and o_proj can overlap on separate banks.
                with tc.tile_pool(name="sc", bufs=1, space="PSUM") as scp, \
                     tc.tile_pool(name="yt", bufs=1, space="PSUM") as ytp:
                    # ---- phase 1: projections + rope + transpose + V ----
                    if 1 in phases:
                        with tc.tile_pool(name="pp", bufs=1, space="PSUM") as pp:
                            ones_set = False
                            for m in range(SB):
                                ms = slice(m * 128, (m + 1) * 128)
                                # QK projection: [128 s, 512] = x_m @ [Wq|Wk]
                                ps = pp.tile([128, 2 * GDIM], F32, tag="ps_qk")
                                for k in range(KD):
                                    nc.tensor.matmul(ps[:], xt[:, k, ms], wqk[:, k, :],
                                                     start=(k == 0), stop=(k == KD - 1))
                                qkr = ropet.tile([128, 2 * GDIM], BF16, tag="qkr")
                                if use_rope:
                                    # single fast cast-copy releases the psum
                                    # slot; rope runs in bf16 on SBUF (2x DVE)
                                    qkf = ropet.tile([128, 2 * GDIM], BF16,
                                                     tag="qkf")
                                    nc.vector.tensor_copy(qkf[:], ps[:])
                                    pv = qkf.rearrange("p (x two) -> p two x", two=2)
                                    ov = qkr.rearrange("p (x two) -> p two x", two=2)
                                    E, O = pv[:, 0, :], pv[:, 1, :]
                                    C, Sn = cos8[:, m, :], sin8[:, m, :]
                                    ta = ropet.tile([128, 256], BF16, tag="ta")
                                    tb = ropet.tile([128, 256], BF16, tag="tb")
                                    nc.vector.tensor_mul(ta[:], E, C)
                                    nc.vector.tensor_mul(tb[:], O, Sn)
                                    nc.vector.tensor_sub(ov[:, 0, :], ta[:], tb[:])
                                    tc_ = ropet.tile([128, 256], BF16, tag="tc")
                                    td = ropet.tile([128, 256], BF16, tag="td")
                                    nc.vector.tensor_mul(tc_[:], O, C)
                                    nc.vector.tensor_mul(td[:], E, Sn)
                                    nc.vector.tensor_add(ov[:, 1, :], tc_[:], td[:])
                                else:
                                    nc.vector.tensor_copy(qkr[:], ps[:])
                                # transpose 128x128 blocks into qkt tiles
                                for cb in range(4):
                                    nc.sync.dma_start_transpose(
                                        qkt[cb][:, ms], qkr[:, cb * 128:(cb + 1) * 128])

                                # V projection: [128 s, 256]
                                psv = pp.tile([128, GDIM], F32, tag="ps_v")
                                for k in range(KD):
                                    nc.tensor.matmul(psv[:], xt[:, k, ms], wv[:, k, :],
                                                     start=(k == 0), stop=(k == KD - 1))
                                if not ones_set:
                                    nc.vector.memset(vsb[:], 1.0)
                                    ones_set = True
                                # copy 4 head blocks of 64 into stride-65 slots
                                dst = vsb[:, m, :].rearrange("p (h c) -> p h c", h=4)[:, :, 0:64]
                                src = psv.rearrange("p (h c) -> p h c", h=4)
                                nc.vector.tensor_copy(dst, src)

                    # ---- phase 2: attention, head pairs row-packed on PE ----
                    if 2 in phases:
                        # Heads 2p and 2p+1 share qkt tiles (partitions 0-63 / 64-127);
                        # their scoresT matmuls are issued to PE row groups 0 and 64 via
                        # tile_position auto-derivation and run concurrently.
                        for qc in range(NQC):
                            for hp in range(2):
                                qt = qkt[hp]
                                kt = qkt[2 + hp]
                                q0 = qc * QCHUNK
                                # Pack kb blocks into wide psum tiles of
                                # WIDE cols. A matmul may not cross a 512-col
                                # psum bank, so emit widths in order
                                # 512,...,512,384,128,256 (384+128=512 tiles
                                # banks exactly; 256 trails).
                                order = list(range(4 * qc)) + \
                                    [4 * qc, 4 * qc + 1, 4 * qc + 3, 4 * qc + 2]
                                groups, cur = [], []
                                cols = 0
                                for kb in order:
                                    r = max(0, kb - 4 * qc)
                                    qoff, n = q0 + r * 128, QCHUNK - r * 128
                                    if cols + n > WIDE:
                                        groups.append(cur)
                                        cur, cols = [], 0
                                    cur.append((kb, qoff, n, cols))
                                    cols += n
                                groups.append(cur)
                                last_kb = groups[-1][-1][0]


                                ytps = [ytp.tile([65, QCHUNK], F32,
                                                 tag=f"ytps{i}", name=f"ytps{i}")
                                        for i in range(2)]
                                for grp in groups:
                                    gcols = grp[-1][3] + grp[-1][2]
                                    scs = [scp.tile([128, WIDE], F32, tag=f"sc{i}",
                                                    name=f"sc{i}") for i in range(2)]
                                    for i in range(2):
                                        rows = slice(i * 64, i * 64 + 64)
                                        for (kb, qoff, n, o) in grp:
                                            nc.tensor.matmul(
                                                scs[i][:, o:o + n],
                                                kt[rows, kb * 128:(kb + 1) * 128],
                                                qt[rows, qoff:qoff + n],
                                                start=True, stop=True)
                                    for i in range(2):
                                        h = 2 * hp + i
                                        vcol = slice(h * 65, h * 65 + 65)
                                        pe = pex.tile([128, WIDE], BF16,
                                                      tag=f"pe{i}", name=f"pe{i}")
                                        nc.scalar.activation(pe[:, :gcols],
                                                             scs[i][:, :gcols],
                                                             EXP, scale=0.125)
                                        for (kb, qoff, n, o) in grp:
                                            if kb >= 4 * qc:  # diagonal: causal mask
                                                nc.vector.tensor_mul(
                                                    pe[:, o:o + 128], pe[:, o:o + 128],
                                                    maskT[:])
                                            # kb==0 always has n=512: start clears
                                            # the whole [65, QCHUNK] accumulator
                                            nc.tensor.matmul(
                                                ytps[i][:, qoff - q0:qoff - q0 + n],
                                                vsb[:, kb, vcol],
                                                pe[:, o:o + n],
                                                start=(kb == 0), stop=(kb == last_kb))
                                for i in range(2):
                                    h = 2 * hp + i
                                    # single copy releases the psum bank for
                                    # the next chunk's PV; normalize from SBUF
                                    ytu = work.tile([65, QCHUNK], F32, tag="ytu")
                                    nc.vector.tensor_copy(ytu[:], ytps[i][:])
                                    rc = work.tile([1, QCHUNK], F32, tag="rc")
                                    nc.vector.reciprocal(rc[:], ytu[64:65, :])
                                    bc = work.tile([64, QCHUNK], F32, tag="bc")
                                    nc.gpsimd.partition_broadcast(bc[:], rc[0:1, :])
                                    nc.vector.tensor_mul(
                                        yt2[hp][i * 64:i * 64 + 64, q0:q0 + QCHUNK],
                                        ytu[0:64, :], bc[:])

                    # ---- phase 3: o_proj ----
                    if 3 in phases:
                        with tc.tile_pool(name="op", bufs=2, space="PSUM") as op:
                            for m in range(SB):
                                ms = slice(m * 128, (m + 1) * 128)
                                for nb in range(2):
                                    po = op.tile([128, 512], F32, tag="po")
                                    for k in range(2):
                                        nc.tensor.matmul(po[:], yt2[k][:, ms],
                                                         wo[:, k, nb * 512:(nb + 1) * 512],
                                                         start=(k == 0), stop=(k == 1))
                                    so = work.tile([128, 512], F32, tag="so")
                                    nc.vector.tensor_copy(so[:], po[:])
                                    nc.sync.dma_start(
                                        out_d[ms, nb * 512:(nb + 1) * 512], so[:])
                                    if timing and out_small is not None and m == 0 and nb == 0:
                                        nc.sync.dma_start(out_small[:], so[:])
    nc.compile()
    return nc


def _prep_core_inputs(x, Wq, Wk, Wv, Wo, cos_g, sin_g, use_rope):
    """Host-side shard + layout prep. Returns list of 8 input dicts."""
    maskT = np.tril(np.ones((128, 128), np.float32)).T.astype(_BF16)
    # interleave cos/sin to the 256-wide repeating pattern used by rope
    cos8 = np.tile(cos_g, (1, 8)).astype(_BF16)
    sin8 = np.tile(sin_g, (1, 8)).astype(_BF16)
    maps = []
    for c in range(NCORES):
        b, g = divmod(c, HEADS_PER_CORE)
        rows = slice(g * GDIM, (g + 1) * GDIM)
        wqk = np.concatenate([Wq[rows], Wk[rows]], axis=0).T  # [D, 512]
        maps.append({
            "xt": np.ascontiguousarray(x[b].T).astype(_BF16),
            "wqk": np.ascontiguousarray(wqk).astype(_BF16),
            "wv": np.ascontiguousarray(Wv[rows].T).astype(_BF16),
            "wo": np.ascontiguousarray(Wo[:, rows].T).astype(_BF16),
            "cos8": cos8,
            "sin8": sin8,
            "maskT": maskT,
        })
    return maps


def kernel(x, token_positions, use_rope, Wq, Wk, Wv, Wo, cos, sin):
    from concourse.bass_utils import run_bass_kernel_spmd

    x = np.asarray(x, np.float32)
    token_positions = np.asarray(token_positions)
    Wq = np.asarray(Wq, np.float32)
    Wk = np.asarray(Wk, np.float32)
    Wv = np.asarray(Wv, np.float32)
    Wo = np.asarray(Wo, np.float32)
    cos = np.asarray(cos, np.float32)
    sin = np.asarray(sin, np.float32)
    rope = bool(int(use_rope))

    cos_g = cos[token_positions]  # [S, 32]
    sin_g = sin[token_positions]

    if rope not in _cache:
        _cache[rope] = _build(rope)
    nc = _cache[rope]

    in_maps = _prep_core_inputs(x, Wq, Wk, Wv, Wo, cos_g, sin_g, rope)
    res = run_bass_kernel_spmd(nc, in_maps, list(range(NCORES)))

    out = np.zeros((B, S, D), np.float32)
    for c in range(NCORES):
        out[c // HEADS_PER_CORE] += res.results[c]["out"]
    return out



# revision 25
# speedup vs baseline: 1.2248x; 1.2248x over previous
"""Causal multi-head attention with RoPE for Trainium2, 8-core SPMD.

Problem: B=2, S=2048, D_MODEL=1024, H=16, HD=64, causal softmax(QK^T/8)V
with interleaved-pair RoPE on q/k, projections Wq/Wk/Wv/Wo.

Sharding (host side): batch x head-group. Core c handles batch b=c//4 and
head group g=c%4 (heads 4g..4g+3, a 256-wide slice of the projection dims).
Each core computes a full [S, D_MODEL] partial of the output (its head
group's contribution through Wo); host sums 4 partials per batch.

Device strategy (all matmuls bf16, fp32 accumulate):
 - emission interleaves projection m-tile chunks with attention q-chunks and
   o_proj chunks so the PE stream never waits on a phase boundary; input DMAs
   are split into chunks so the first projection starts ~5us in
 - host permutes Wq/Wk rows per head to [evens | odds] so RoPE operates on
   contiguous 32-col blocks (packed DVE/Pool ops); scores are invariant since
   q and k share the permutation
 - Q,K projected in [s, o] layout -> RoPE (4 muls on Pool, add/sub on DVE)
   -> one batched DMA transpose per m-tile into qkt4 [128, 4, S]
 - scoresT[k, q] = Kt.T @ Qt per 128-key block, head pairs row-packed on PE
   partitions 0:64/64:128; wide [128, 1024] PSUM score tiles, one Exp per
   (group, head) on ACT; causal diagonal masked by Pool multiply
 - PV flipped: out[q, h, hd] with lhsT = probs block [keys, q], rhs =
   [V | 1] [keys, 65] -- N=65 per matmul so PV costs 65 cycles/block instead
   of 128+; col 64 accumulates the softmax denominator per q partition, so
   normalization is reciprocal + per-partition tensor_scalar
 - y [q, hd] normalized then batch-transposed into yt2 [128, 2, S];
   o_proj per q-chunk, PSUM evacuated by DVE, one output DMA per m-tile
"""

import numpy as np
import ml_dtypes

B, S, D, H = 2, 2048, 1024, 16
HD = 64
NCORES = 8
HEADS_PER_CORE = 4
GDIM = HEADS_PER_CORE * HD          # 256 projection cols per core
SB = S // 128                        # 16 s-tiles
KD = D // 128                        # 8 k-tiles over d
QCHUNK = 512
NQC = S // QCHUNK                    # 4 q-chunks
WIDE = 1024                          # wide scores psum tile (2 banks)

_BF16 = ml_dtypes.bfloat16
_cache = {}


def _score_layout(qc):
    """Per (qc): list of (kb, qoff, n) in emission order and the global column
    base of each kb block in the pe probs buffer; plus chunking into <=WIDE
    score-psum groups. Returns (groups, base) where groups is a list of
    [(kb, qoff, n, colbase), ...] and base maps kb -> global pe column."""
    q0 = qc * QCHUNK
    order = list(range(4 * qc)) + [4 * qc, 4 * qc + 1, 4 * qc + 3, 4 * qc + 2]
    base = {}
    blocks = []
    pos = 0
    for kb in order:
        r = max(0, kb - 4 * qc)
        qoff = q0 + r * 128 if kb >= 4 * qc else q0
        n = QCHUNK - r * 128 if kb >= 4 * qc else QCHUNK
        base[kb] = pos
        blocks.append((kb, qoff, n, pos))
        pos += n
    groups, cur, cols = [], [], 0
    for (kb, qoff, n, colbase) in blocks:
        if cols + n > WIDE:
            groups.append(cur)
            cur, cols = [], 0
        cur.append((kb, qoff, n, colbase))
        cols += n
    groups.append(cur)
    return groups, base, pos


def _build(use_rope: bool):
    import concourse.bass as bass
    import concourse.mybir as mybir
    import concourse.tile as tile
    from concourse import bacc
    from contextlib import ExitStack

    F32 = mybir.dt.float32
    BF16 = mybir.dt.bfloat16
    EXP = mybir.ActivationFunctionType.Exp
    MULT = mybir.AluOpType.mult

    nc = bacc.Bacc(None, target_bir_lowering=False)

    xt_d = nc.dram_tensor("xt", [D, S], BF16, kind="ExternalInput")
    wqk_d = nc.dram_tensor("wqk", [D, 2 * GDIM], BF16, kind="ExternalInput")
    wv_d = nc.dram_tensor("wv", [D, GDIM], BF16, kind="ExternalInput")
    wo_d = nc.dram_tensor("wo", [GDIM, D], BF16, kind="ExternalInput")
    cos_d = nc.dram_tensor("cos16", [S, 512], BF16, kind="ExternalInput")
    sin_d = nc.dram_tensor("sin16", [S, 512], BF16, kind="ExternalInput")
    mask_d = nc.dram_tensor("maskT", [128, 128], BF16, kind="ExternalInput")
    out_d = nc.dram_tensor("out", [S, D], F32, kind="ExternalOutput")

    # pe probs buffer column count for the widest chunk (qc=3)
    _, _, NCOLS = _score_layout(NQC - 1)

    xt_dr = xt_d.rearrange("(k p) s -> p k s", p=128)
    wqk_dr = wqk_d.rearrange("(k p) o -> p k o", p=128)
    cos_dr = cos_d.rearrange("(m p) f -> p m f", p=128)
    sin_dr = sin_d.rearrange("(m p) f -> p m f", p=128)

    with tile.TileContext(nc) as tc:
        es = ExitStack()
        big = es.enter_context(tc.tile_pool(name="big", bufs=1))
        work = es.enter_context(tc.tile_pool(name="work", bufs=2))
        scp = es.enter_context(tc.tile_pool(name="sc", bufs=1, space="PSUM"))
        yqp = es.enter_context(tc.tile_pool(name="yq", bufs=2, space="PSUM"))

        # ---- resident tiles ----
        wo = big.tile([128, 2, D], BF16)
        maskT = big.tile([128, 128], BF16)
        qkt4 = big.tile([128, 4, S], BF16)
        vsb = big.tile([128, SB, HEADS_PER_CORE * 65], BF16)
        yt2 = big.tile([128, 2, S], BF16)
        # probs buffers for qc0-2 (max 5376 cols); qc3 gets its own buffers
        # carved out of the released phase-1 pool so exp(qc3) need not wait
        # for PV(qc2) to drain these
        _, _, NC2 = _score_layout(2)
        _, _, NC1 = _score_layout(1)
        pe_main = [[big.tile([128, NC2], BF16, tag=f"pe{hp}{i}",
                             name=f"pe{hp}{i}") for i in range(2)]
                   for hp in range(2)]
        # phase-1-only tensors: released after the last projection m-tile
        ph1_ctx = tc.tile_pool(name="ph1", bufs=1)
        ph1 = ph1_ctx.__enter__()
        xt = ph1.tile([128, KD, S], BF16)
        wqk = ph1.tile([128, KD, 2 * GDIM], BF16)
        wv = ph1.tile([128, KD, GDIM], BF16)
        if use_rope:
            cos16 = ph1.tile([128, SB, 512], BF16)
            sin16 = ph1.tile([128, SB, 512], BF16)

        # ones columns of [V | 1] (memset before anything else)
        vsb4 = vsb.rearrange("p m (h c) -> p m h c", h=4)
        nc.vector.memset(vsb4[:, :, :, 64:65], 1.0)

        # ---- input DMAs, chunked so m-tile 0 unblocks early; weights go
        # down the ACT queue in parallel with xt on the SP queue ----
        nc.sync.dma_start(wqk[:, 0:2, :], wqk_dr[:, 0:2, :])
        nc.sync.dma_start(xt[:, 0:2, 0:QCHUNK], xt_dr[:, 0:2, 0:QCHUNK])
        nc.sync.dma_start(wqk[:, 2:4, :], wqk_dr[:, 2:4, :])
        nc.sync.dma_start(xt[:, 2:4, 0:QCHUNK], xt_dr[:, 2:4, 0:QCHUNK])
        nc.sync.dma_start(wqk[:, 4:8, :], wqk_dr[:, 4:8, :])
        nc.sync.dma_start(xt[:, 4:8, 0:QCHUNK], xt_dr[:, 4:8, 0:QCHUNK])
        nc.sync.dma_start(wv[:], wv_d.rearrange("(k p) o -> p k o", p=128))
        if use_rope:
            nc.sync.dma_start(cos16[:, 0:4, :], cos_dr[:, 0:4, :])
            nc.sync.dma_start(sin16[:, 0:4, :], sin_dr[:, 0:4, :])
        nc.sync.dma_start(maskT[:], mask_d[:])
        for c in range(1, 4):
            cs = slice(c * QCHUNK, (c + 1) * QCHUNK)
            nc.sync.dma_start(xt[:, :, cs], xt_dr[:, :, cs])
            if use_rope:
                nc.sync.dma_start(cos16[:, 4*c:4*c+4, :], cos_dr[:, 4*c:4*c+4, :])
                nc.sync.dma_start(sin16[:, 4*c:4*c+4, :], sin_dr[:, 4*c:4*c+4, :])
        nc.sync.dma_start(wo[:], wo_d.rearrange("(k p) o -> p k o", p=128))

        # ---------- emission helpers ----------
        def proj_mtile(m):
            """QKV projection + rope + transpose + V staging for s-tile m."""
            ms = slice(m * 128, (m + 1) * 128)
            ps = pp.tile([128, 2 * GDIM], F32, tag="ps_qk", name="ps")
            for k in range(KD):
                nc.tensor.matmul(ps[:], xt[:, k, ms], wqk[:, k, :],
                                 start=(k == 0), stop=(k == KD - 1))
            qkr = work.tile([128, 2 * GDIM], BF16, tag="qkr", name="qkr")
            if use_rope:
                qkf = work.tile([128, 2 * GDIM], BF16, tag="qkf", name="qkf")
                if m < 4:
                    nc.scalar.copy(qkf[:], ps[:])
                else:
                    nc.vector.tensor_copy(qkf[:], ps[:])
                # head dims are [evens(32) | odds(32)] per 64-block (host
                # permuted).  Two full-width muls compute all four products:
                # t_c = qkf*cos = [E*c | O*c], t_s = qkf*sin = [E*s | O*s]
                ov = qkr.rearrange("p (hb eo f) -> p hb eo f", eo=2, f=32)
                t_c = work.tile([128, 512], BF16, tag="tc", name="tc")
                t_s = work.tile([128, 512], BF16, tag="ts", name="ts")
                nc.vector.tensor_mul(t_c[:], qkf[:], cos16[:, m, :])
                nc.vector.tensor_mul(t_s[:], qkf[:], sin16[:, m, :])
                tcv = t_c.rearrange("p (hb eo f) -> p hb eo f", eo=2, f=32)
                tsv = t_s.rearrange("p (hb eo f) -> p hb eo f", eo=2, f=32)
                # e' = E*c - O*s ; o' = O*c + E*s
                nc.vector.tensor_sub(ov[:, :, 0, :], tcv[:, :, 0, :], tsv[:, :, 1, :])
                nc.vector.tensor_add(ov[:, :, 1, :], tcv[:, :, 1, :], tsv[:, :, 0, :])
            else:
                nc.vector.tensor_copy(qkr[:], ps[:])
            # one batched transpose: [128 s, 512 o] -> qkt4[:, 0:4, m-block]
            gms = slice(m * 128, (m + 1) * 128)
            nc.sync.dma_start_transpose(qkt4[:, :, gms], qkr[:])

            psv = pp.tile([128, GDIM], F32, tag="ps_v", name="psv")
            for k in range(KD):
                nc.tensor.matmul(psv[:], xt[:, k, ms], wv[:, k, :],
                                 start=(k == 0), stop=(k == KD - 1))
            dst = vsb4[:, m, :, 0:64]
            src = psv.rearrange("p (h c) -> p h c", h=4)
            if m < 4:
                nc.scalar.copy(dst, src)
            else:
                nc.vector.tensor_copy(dst, src)

        def attention_scores_hp(qc, hp, pe_all):
            """Scores + exp + causal mask for one head pair of q-chunk qc."""
            groups, base, ncols = _score_layout(qc)
            if True:
                qt = qkt4[:, hp, :]
                kt = qkt4[:, 2 + hp, :]
                for grp in groups:
                    gbase = grp[0][3]
                    gcols = grp[-1][3] + grp[-1][2] - gbase
                    scs = [scp.tile([128, WIDE], F32, tag=f"sc{i}",
                                    name=f"sc{i}") for i in range(2)]
                    for i in range(2):
                        rows = slice(i * 64, i * 64 + 64)
                        for (kb, qoff, n, colbase) in grp:
                            o = colbase - gbase
                            nc.tensor.matmul(
                                scs[i][:, o:o + n],
                                kt[rows, kb * 128:(kb + 1) * 128],
                                qt[rows, qoff:qoff + n],
                                start=True, stop=True)
                    for i in range(2):
                        pe = pe_all[hp][i]
                        nc.scalar.activation(pe[:, gbase:gbase + gcols],
                                             scs[i][:, :gcols], EXP, scale=0.125)
                        for (kb, qoff, n, colbase) in grp:
                            if kb >= 4 * qc:  # diagonal block: causal mask
                                nc.gpsimd.tensor_mul(
                                    pe[:, colbase:colbase + 128],
                                    pe[:, colbase:colbase + 128], maskT[:])

        def attention_pv(qc, pe_all, qls=(0, 1, 2, 3)):
            """Flipped PV per q-block: out [128 q, 4 heads, 65], then
            normalize via the accumulated denominator column + transpose."""
            _, base, _ = _score_layout(qc)
            for ql in qls:
                qb = 4 * qc + ql
                yq = yqp.tile([128, 4, 65], F32, tag="yq", name="yq")
                for h in range(4):
                    hp, i = divmod(h, 2)
                    pe = pe_all[hp][i]
                    for kb in range(qb + 1):
                        off = 128 * ql if kb < 4 * qc else 128 * (qb - kb)
                        col = base[kb] + off
                        nc.tensor.matmul(
                            yq[:, h, :], pe[:, col:col + 128],
                            vsb[:, kb, h * 65:(h + 1) * 65],
                            start=(kb == 0), stop=(kb == qb))
                yq_sb = work.tile([128, 4, 65], F32, tag="yqsb", name="yqsb")
                if qc == 0:
                    nc.scalar.copy(yq_sb[:], yq[:])
                else:
                    nc.vector.tensor_copy(yq_sb[:], yq[:])
                rc = work.tile([128, 4], F32, tag="rc", name="rc")
                nc.vector.reciprocal(rc[:], yq_sb[:, :, 64])
                y_sb = work.tile([128, 4, 64], BF16, tag="ysb", name="ysb")
                for h in range(4):
                    nc.gpsimd.tensor_scalar(y_sb[:, h, :], yq_sb[:, h, 0:64],
                                            rc[:, h:h + 1], None, MULT)
                nc.sync.dma_start_transpose(
                    yt2[:, :, qb * 128:(qb + 1) * 128], y_sb[:])

        def oproj_m(m, tags=("ps_qk", "ps_v"), evac=("dve", "dve"), out_q="sp",
                    split_out=False):
            # po reuses the phase-1 projection PSUM banks (tags rotate) --
            # avoids a pool boundary, which would order o_proj after every
            # phase-1 instruction.  After the last exp, the sc tags can join
            # the rotation for a deeper po pipeline.
            ms = slice(m * 128, (m + 1) * 128)
            so = work.tile([128, D], F32, tag="so", name="so", bufs=4)
            for nb in range(2):
                if tags[nb] in ("ps_qk", "ps_v"):
                    po = pp.tile([128, 512], F32, tag=tags[nb], name="po")
                elif tags[nb] == "yq":
                    po = yqp.tile([128, 512], F32, tag="yq", name="po")
                else:
                    po = scp.tile([128, WIDE], F32, tag=tags[nb], name="po")
                for k in range(2):
                    nc.tensor.matmul(po[:, 0:512], yt2[:, k, ms],
                                     wo[:, k, nb * 512:(nb + 1) * 512],
                                     start=(k == 0), stop=(k == 1))
                dst = so[:, nb * 512:(nb + 1) * 512]
                if evac[nb] == "dve":
                    nc.vector.tensor_copy(dst, po[:, 0:512])
                else:
                    nc.scalar.copy(dst, po[:, 0:512])
                if split_out:
                    eng = nc.scalar if (m + nb) % 2 == 0 else nc.sync
                    eng.dma_start(out_d[ms, nb * 512:(nb + 1) * 512], dst)
            if not split_out:
                if out_q == "sp":
                    nc.sync.dma_start(out_d[ms, :], so[:])
                else:
                    nc.scalar.dma_start(out_d[ms, :], so[:])

        # ---------- interleaved emission ----------
        # Fine-grained round-robin: each score-group's exp (ACT) is shadowed
        # by a projection m-tile (PE) so the PE stream never blocks on the
        # single-buffered score PSUM tiles.
        pp = es.enter_context(tc.tile_pool(name="pp", bufs=1, space="PSUM"))
        for m in range(0, 4):
            proj_mtile(m)
        attention_scores_hp(0, 0, pe_main)
        proj_mtile(4)
        attention_scores_hp(0, 1, pe_main)
        proj_mtile(5)
        proj_mtile(6)
        proj_mtile(7)
        attention_pv(0, pe_main)
        attention_scores_hp(1, 0, pe_main)
        proj_mtile(8)
        attention_scores_hp(1, 1, pe_main)
        proj_mtile(9)
        proj_mtile(10)
        proj_mtile(11)
        attention_pv(1, pe_main)
        attention_scores_hp(2, 0, pe_main)
        proj_mtile(12)
        attention_scores_hp(2, 1, pe_main)
        proj_mtile(13)
        proj_mtile(14)
        proj_mtile(15)
        # phase 1 done: free xt/w/cos/sin, carve qc3 probs buffers from the
        # freed region so exp(qc3) is independent of PV(qc2)
        ph1_ctx.__exit__(None, None, None)
        with tc.tile_pool(name="pe3p", bufs=1) as pe3p:
            pe3 = [[pe3p.tile([128, NCOLS], BF16, tag=f"pe3{hp}{i}",
                              name=f"pe3{hp}{i}") for i in range(2)]
                   for hp in range(2)]
            attention_scores_hp(3, 0, pe3)
            attention_pv(2, pe_main)
            for m in range(0, 4):
                oproj_m(m)
            attention_scores_hp(3, 1, pe3)
            for m in range(4, 8):
                oproj_m(m)
            oproj_m(8, out_q="act")
            oproj_m(9, out_q="sp")
            oproj_m(10, out_q="act")
            oproj_m(11, out_q="sp")
            # tail: all four PV chains first (their normalize->transpose
            # chains pipeline down DVE/Pool/SP while PE works), then the
            # last o_proj tiles with po rotating through 4 banks and out
            # DMAs alternating between the SP and ACT queues
            attention_pv(3, pe3)
            oproj_m(12, tags=("ps_qk", "ps_v"), evac=("dve", "act"), out_q="act")
            oproj_m(13, tags=("sc0", "sc1"), evac=("dve", "act"), out_q="sp")
            oproj_m(14, tags=("ps_qk", "ps_v"), evac=("dve", "act"), out_q="act")
            oproj_m(15, tags=("yq", "sc0"), evac=("dve", "act"), out_q="sp")
        es.close()
    nc.compile()
    return nc


_PERM64 = np.concatenate([np.arange(0, 64, 2), np.arange(1, 64, 2)])


def _prep_core_inputs(x, Wq, Wk, Wv, Wo, cos_g, sin_g, use_rope):
    """Host-side shard + layout prep. Returns list of 8 input dicts."""
    maskT = np.tril(np.ones((128, 128), np.float32)).T.astype(_BF16)
    # 16 copies of the 32-wide tables: [evens|odds] per head block x 8 blocks
    cos16 = np.tile(cos_g, (1, 16)).astype(_BF16)
    sin16 = np.tile(sin_g, (1, 16)).astype(_BF16)
    maps = []
    for c in range(NCORES):
        b, g = divmod(c, HEADS_PER_CORE)
        rows = slice(g * GDIM, (g + 1) * GDIM)
        wq_g = Wq[rows]
        wk_g = Wk[rows]
        if use_rope:
            # per-head row permutation to [evens(32) | odds(32)] so device
            # rope works on contiguous blocks; scores invariant (q,k share it)
            wq_g = wq_g.reshape(HEADS_PER_CORE, HD, D)[:, _PERM64, :].reshape(GDIM, D)
            wk_g = wk_g.reshape(HEADS_PER_CORE, HD, D)[:, _PERM64, :].reshape(GDIM, D)
        wqk = np.concatenate([wq_g, wk_g], axis=0).T  # [D, 512]
        maps.append({
            "xt": np.ascontiguousarray(x[b].T).astype(_BF16),
            "wqk": np.ascontiguousarray(wqk).astype(_BF16),
            "wv": np.ascontiguousarray(Wv[rows].T).astype(_BF16),
            "wo": np.ascontiguousarray(Wo[:, rows].T).astype(_BF16),
            "cos16": cos16,
            "sin16": sin16,
            "maskT": maskT,
        })
    return maps


def kernel(x, token_positions, use_rope, Wq, Wk, Wv, Wo, cos, sin):
    from concourse.bass_utils import run_bass_kernel_spmd

    x = np.asarray(x, np.float32)
    token_positions = np.asarray(token_positions)
    Wq = np.asarray(Wq, np.float32)
    Wk = np.asarray(Wk, np.float32)
    Wv = np.asarray(Wv, np.float32)
    Wo = np.asarray(Wo, np.float32)
    cos = np.asarray(cos, np.float32)
    sin = np.asarray(sin, np.float32)
    rope = bool(int(use_rope))

    cos_g = cos[token_positions]  # [S, 32]
    sin_g = sin[token_positions]

    if rope not in _cache:
        _cache[rope] = _build(rope)
    nc = _cache[rope]

    in_maps = _prep_core_inputs(x, Wq, Wk, Wv, Wo, cos_g, sin_g, rope)
    res = run_bass_kernel_spmd(nc, in_maps, list(range(NCORES)))

    out = np.zeros((B, S, D), np.float32)
    for c in range(NCORES):
        out[c // HEADS_PER_CORE] += res.results[c]["out"]
    return out


# revision 30
# speedup vs baseline: 1.2745x; 1.0406x over previous
"""Causal multi-head attention with RoPE for Trainium2, 8-core SPMD.

Problem: B=2, S=2048, D_MODEL=1024, H=16, HD=64, causal softmax(QK^T/8)V
with interleaved-pair RoPE on q/k, projections Wq/Wk/Wv/Wo.

Sharding (host side): batch x head-group. Core c handles batch b=c//4 and
head group g=c%4 (heads 4g..4g+3, a 256-wide slice of the projection dims).
Each core computes a full [S, D_MODEL] partial of the output (its head
group's contribution through Wo) in bf16; host casts to f32 and sums 4
partials per batch.

Device strategy (all matmuls bf16, fp32 accumulate):
 - emission interleaves projection m-tiles, attention q-chunks and o_proj
   tiles at score-group granularity so the PE stream always has work while
   ACT chews through the exp backlog; input DMAs are chunked so the first
   projection starts ~4.5us in
 - host permutes Wq/Wk rows per head to [evens(32) | odds(32)] so RoPE
   reads contiguous blocks: two full-width muls (qkf*cos16, qkf*sin16) +
   strided-block add/sub on DVE; scores are invariant to the permutation
   since q and k share it
 - Q,K projected in [s, o] layout -> RoPE -> one batched DMA transpose per
   m-tile into qkt4 [128, 4, S]; QK projection PSUM double-buffered across
   two tags, V accumulates via the PV psum ring
 - scoresT[k, q] = Kt.T @ Qt per 128-key block, head pairs row-packed on PE
   partitions 0:64/64:128; wide [128, 1024] PSUM score tiles, one Exp per
   (group, head) on ACT writing probs into per-(hp,head) SBUF buffers
   (qc0-2 share one set; qc3 gets its own carved from the released
   phase-1 pool so exp(qc3) never waits on PV(qc2)); causal diagonal
   masked by Pool multiply
 - PV flipped: out[q, h, hd] with lhsT = probs block [keys, q], rhs =
   [V | 1] [keys, 65] -- N=65 per matmul (the cost driver is the moving
   dim) instead of 128-512; col 64 accumulates the softmax denominator per
   q partition, so normalization is one reciprocal + per-partition
   tensor_scalar on Pool
 - y [q, hd] normalized then batch-transposed into yt2 [128, 2, S];
   o_proj per q-chunk with po PSUM rotating over freed phase-1/score
   banks, evacuation split DVE/ACT at the tail, out DMAs in bf16
   alternating between the SP and ACT DMA queues
"""

import numpy as np
import ml_dtypes

B, S, D, H = 2, 2048, 1024, 16
HD = 64
NCORES = 8
HEADS_PER_CORE = 4
GDIM = HEADS_PER_CORE * HD          # 256 projection cols per core
SB = S // 128                        # 16 s-tiles
KD = D // 128                        # 8 k-tiles over d
QCHUNK = 512
NQC = S // QCHUNK                    # 4 q-chunks
WIDE = 1024                          # wide scores psum tile (2 banks)

_BF16 = ml_dtypes.bfloat16
_cache = {}


def _score_layout(qc):
    """Per (qc): list of (kb, qoff, n) in emission order and the global column
    base of each kb block in the pe probs buffer; plus chunking into <=WIDE
    score-psum groups. Returns (groups, base) where groups is a list of
    [(kb, qoff, n, colbase), ...] and base maps kb -> global pe column."""
    q0 = qc * QCHUNK
    order = list(range(4 * qc)) + [4 * qc, 4 * qc + 1, 4 * qc + 3, 4 * qc + 2]
    base = {}
    blocks = []
    pos = 0
    for kb in order:
        r = max(0, kb - 4 * qc)
        qoff = q0 + r * 128 if kb >= 4 * qc else q0
        n = QCHUNK - r * 128 if kb >= 4 * qc else QCHUNK
        base[kb] = pos
        blocks.append((kb, qoff, n, pos))
        pos += n
    groups, cur, cols = [], [], 0
    for (kb, qoff, n, colbase) in blocks:
        if cols + n > WIDE:
            groups.append(cur)
            cur, cols = [], 0
        cur.append((kb, qoff, n, colbase))
        cols += n
    groups.append(cur)
    return groups, base, pos


def _build(use_rope: bool):
    import concourse.bass as bass
    import concourse.mybir as mybir
    import concourse.tile as tile
    from concourse import bacc
    from contextlib import ExitStack

    F32 = mybir.dt.float32
    BF16 = mybir.dt.bfloat16
    EXP = mybir.ActivationFunctionType.Exp
    MULT = mybir.AluOpType.mult

    nc = bacc.Bacc(None, target_bir_lowering=False)

    xt_d = nc.dram_tensor("xt", [D, S], BF16, kind="ExternalInput")
    wqk_d = nc.dram_tensor("wqk", [D, 2 * GDIM], BF16, kind="ExternalInput")
    wv_d = nc.dram_tensor("wv", [D, GDIM], BF16, kind="ExternalInput")
    wo_d = nc.dram_tensor("wo", [GDIM, D], BF16, kind="ExternalInput")
    cos_d = nc.dram_tensor("cos16", [S, 512], BF16, kind="ExternalInput")
    sin_d = nc.dram_tensor("sin16", [S, 512], BF16, kind="ExternalInput")
    mask_d = nc.dram_tensor("maskT", [128, 128], BF16, kind="ExternalInput")
    out_d = nc.dram_tensor("out", [S, D], BF16, kind="ExternalOutput")

    # pe probs buffer column count for the widest chunk (qc=3)
    _, _, NCOLS = _score_layout(NQC - 1)

    xt_dr = xt_d.rearrange("(k p) s -> p k s", p=128)
    wqk_dr = wqk_d.rearrange("(k p) o -> p k o", p=128)
    cos_dr = cos_d.rearrange("(m p) f -> p m f", p=128)
    sin_dr = sin_d.rearrange("(m p) f -> p m f", p=128)

    with tile.TileContext(nc) as tc:
        es = ExitStack()
        big = es.enter_context(tc.tile_pool(name="big", bufs=1))
        work = es.enter_context(tc.tile_pool(name="work", bufs=2))
        scp = es.enter_context(tc.tile_pool(name="sc", bufs=1, space="PSUM"))
        yqp = es.enter_context(tc.tile_pool(name="yq", bufs=2, space="PSUM"))

        # ---- resident tiles ----
        wo = big.tile([128, 2, D], BF16)
        maskT = big.tile([128, 128], BF16)
        qkt4 = big.tile([128, 4, S], BF16)
        vsb = big.tile([128, SB, HEADS_PER_CORE * 65], BF16)
        yt2 = big.tile([128, 2, S], BF16)
        # probs buffers for qc0-2 (max 5376 cols); qc3 gets its own buffers
        # carved out of the released phase-1 pool so exp(qc3) need not wait
        # for PV(qc2) to drain these
        _, _, NC2 = _score_layout(2)
        _, _, NC1 = _score_layout(1)
        pe_main = [[big.tile([128, NC2], BF16, tag=f"pe{hp}{i}",
                             name=f"pe{hp}{i}") for i in range(2)]
                   for hp in range(2)]
        # phase-1-only tensors: released after the last projection m-tile
        ph1_ctx = tc.tile_pool(name="ph1", bufs=1)
        ph1 = ph1_ctx.__enter__()
        xt = ph1.tile([128, KD, S], BF16)
        wqk = ph1.tile([128, KD, 2 * GDIM], BF16)
        wv = ph1.tile([128, KD, GDIM], BF16)
        if use_rope:
            cos16 = ph1.tile([128, SB, 512], BF16)
            sin16 = ph1.tile([128, SB, 512], BF16)

        # ones columns of [V | 1] (memset before anything else)
        vsb4 = vsb.rearrange("p m (h c) -> p m h c", h=4)
        nc.vector.memset(vsb4[:, :, :, 64:65], 1.0)

        # ---- input DMAs, chunked so m-tile 0 unblocks early; weights go
        # down the ACT queue in parallel with xt on the SP queue ----
        nc.sync.dma_start(wqk[:, 0:2, :], wqk_dr[:, 0:2, :])
        nc.sync.dma_start(xt[:, 0:2, 0:QCHUNK], xt_dr[:, 0:2, 0:QCHUNK])
        nc.sync.dma_start(wqk[:, 2:4, :], wqk_dr[:, 2:4, :])
        nc.sync.dma_start(xt[:, 2:4, 0:QCHUNK], xt_dr[:, 2:4, 0:QCHUNK])
        nc.sync.dma_start(wqk[:, 4:8, :], wqk_dr[:, 4:8, :])
        nc.sync.dma_start(xt[:, 4:8, 0:QCHUNK], xt_dr[:, 4:8, 0:QCHUNK])
        nc.sync.dma_start(wv[:], wv_d.rearrange("(k p) o -> p k o", p=128))
        if use_rope:
            nc.sync.dma_start(cos16[:, 0:4, :], cos_dr[:, 0:4, :])
            nc.sync.dma_start(sin16[:, 0:4, :], sin_dr[:, 0:4, :])
        nc.sync.dma_start(maskT[:], mask_d[:])
        for c in range(1, 4):
            cs = slice(c * QCHUNK, (c + 1) * QCHUNK)
            nc.sync.dma_start(xt[:, :, cs], xt_dr[:, :, cs])
            if use_rope:
                nc.sync.dma_start(cos16[:, 4*c:4*c+4, :], cos_dr[:, 4*c:4*c+4, :])
                nc.sync.dma_start(sin16[:, 4*c:4*c+4, :], sin_dr[:, 4*c:4*c+4, :])
        nc.sync.dma_start(wo[:], wo_d.rearrange("(k p) o -> p k o", p=128))

        # ---------- emission helpers ----------
        def proj_mtile(m):
            """QKV projection + rope + transpose + V staging for s-tile m."""
            ms = slice(m * 128, (m + 1) * 128)
            ps = pp.tile([128, 2 * GDIM], F32,
                         tag=("ps_qk", "ps_v")[m % 2], name="ps")
            for k in range(KD):
                nc.tensor.matmul(ps[:], xt[:, k, ms], wqk[:, k, :],
                                 start=(k == 0), stop=(k == KD - 1))
            qkr = work.tile([128, 2 * GDIM], BF16, tag="qkr", name="qkr")
            if use_rope:
                qkf = work.tile([128, 2 * GDIM], BF16, tag="qkf", name="qkf")
                if m < 4:
                    nc.scalar.copy(qkf[:], ps[:])
                else:
                    nc.vector.tensor_copy(qkf[:], ps[:])
                # head dims are [evens(32) | odds(32)] per 64-block (host
                # permuted).  Two full-width muls compute all four products:
                # t_c = qkf*cos = [E*c | O*c], t_s = qkf*sin = [E*s | O*s]
                ov = qkr.rearrange("p (hb eo f) -> p hb eo f", eo=2, f=32)
                t_c = work.tile([128, 512], BF16, tag="tc", name="tc")
                t_s = work.tile([128, 512], BF16, tag="ts", name="ts")
                nc.vector.tensor_mul(t_c[:], qkf[:], cos16[:, m, :])
                nc.vector.tensor_mul(t_s[:], qkf[:], sin16[:, m, :])
                tcv = t_c.rearrange("p (hb eo f) -> p hb eo f", eo=2, f=32)
                tsv = t_s.rearrange("p (hb eo f) -> p hb eo f", eo=2, f=32)
                # e' = E*c - O*s ; o' = O*c + E*s
                nc.vector.tensor_sub(ov[:, :, 0, :], tcv[:, :, 0, :], tsv[:, :, 1, :])
                nc.vector.tensor_add(ov[:, :, 1, :], tcv[:, :, 1, :], tsv[:, :, 0, :])
            else:
                nc.vector.tensor_copy(qkr[:], ps[:])
            # one batched transpose: [128 s, 512 o] -> qkt4[:, 0:4, m-block]
            gms = slice(m * 128, (m + 1) * 128)
            nc.sync.dma_start_transpose(qkt4[:, :, gms], qkr[:])

            psv = yqp.tile([128, GDIM], F32, tag="yq", name="psv")
            for k in range(KD):
                nc.tensor.matmul(psv[:], xt[:, k, ms], wv[:, k, :],
                                 start=(k == 0), stop=(k == KD - 1))
            dst = vsb4[:, m, :, 0:64]
            src = psv.rearrange("p (h c) -> p h c", h=4)
            if m < 4:
                nc.scalar.copy(dst, src)
            else:
                nc.vector.tensor_copy(dst, src)

        def attention_scores_hp(qc, hp, pe_all):
            """Scores + exp + causal mask for one head pair of q-chunk qc."""
            groups, base, ncols = _score_layout(qc)
            if True:
                qt = qkt4[:, hp, :]
                kt = qkt4[:, 2 + hp, :]
                for grp in groups:
                    gbase = grp[0][3]
                    gcols = grp[-1][3] + grp[-1][2] - gbase
                    scs = [scp.tile([128, WIDE], F32, tag=f"sc{i}",
                                    name=f"sc{i}") for i in range(2)]
                    for i in range(2):
                        rows = slice(i * 64, i * 64 + 64)
                        for (kb, qoff, n, colbase) in grp:
                            o = colbase - gbase
                            nc.tensor.matmul(
                                scs[i][:, o:o + n],
                                kt[rows, kb * 128:(kb + 1) * 128],
                                qt[rows, qoff:qoff + n],
                                start=True, stop=True)
                    for i in range(2):
                        pe = pe_all[hp][i]
                        nc.scalar.activation(pe[:, gbase:gbase + gcols],
                                             scs[i][:, :gcols], EXP, scale=0.125)
                        for (kb, qoff, n, colbase) in grp:
                            if kb >= 4 * qc:  # diagonal block: causal mask
                                nc.gpsimd.tensor_mul(
                                    pe[:, colbase:colbase + 128],
                                    pe[:, colbase:colbase + 128], maskT[:])

        def attention_pv(qc, pe_all, qls=(0, 1, 2, 3)):
            """Flipped PV per q-block: out [128 q, 4 heads, 65], then
            normalize via the accumulated denominator column + transpose."""
            _, base, _ = _score_layout(qc)
            for ql in qls:
                qb = 4 * qc + ql
                yq = yqp.tile([128, 4, 65], F32, tag="yq", name="yq")
                for h in range(4):
                    hp, i = divmod(h, 2)
                    pe = pe_all[hp][i]
                    for kb in range(qb + 1):
                        off = 128 * ql if kb < 4 * qc else 128 * (qb - kb)
                        col = base[kb] + off
                        nc.tensor.matmul(
                            yq[:, h, :], pe[:, col:col + 128],
                            vsb[:, kb, h * 65:(h + 1) * 65],
                            start=(kb == 0), stop=(kb == qb))
                yq_sb = work.tile([128, 4, 65], F32, tag="yqsb", name="yqsb")
                if qc == 0:
                    nc.scalar.copy(yq_sb[:], yq[:])
                else:
                    nc.vector.tensor_copy(yq_sb[:], yq[:])
                rc = work.tile([128, 4], F32, tag="rc", name="rc")
                nc.vector.reciprocal(rc[:], yq_sb[:, :, 64])
                y_sb = work.tile([128, 4, 64], BF16, tag="ysb", name="ysb")
                for h in range(4):
                    nc.gpsimd.tensor_scalar(y_sb[:, h, :], yq_sb[:, h, 0:64],
                                            rc[:, h:h + 1], None, MULT)
                nc.sync.dma_start_transpose(
                    yt2[:, :, qb * 128:(qb + 1) * 128], y_sb[:])

        def oproj_m(m, tags=("ps_qk", "ps_v"), evac=("dve", "dve"), out_q="sp",
                    split_out=False):
            # po reuses the phase-1 projection PSUM banks (tags rotate) --
            # avoids a pool boundary, which would order o_proj after every
            # phase-1 instruction.  After the last exp, the sc tags can join
            # the rotation for a deeper po pipeline.
            ms = slice(m * 128, (m + 1) * 128)
            so = work.tile([128, D], BF16, tag="so", name="so", bufs=4)
            for nb in range(2):
                if tags[nb] in ("ps_qk", "ps_v"):
                    po = pp.tile([128, 512], F32, tag=tags[nb], name="po")
                elif tags[nb] == "yq":
                    po = yqp.tile([128, 512], F32, tag="yq", name="po")
                else:
                    po = scp.tile([128, WIDE], F32, tag=tags[nb], name="po")
                for k in range(2):
                    nc.tensor.matmul(po[:, 0:512], yt2[:, k, ms],
                                     wo[:, k, nb * 512:(nb + 1) * 512],
                                     start=(k == 0), stop=(k == 1))
                dst = so[:, nb * 512:(nb + 1) * 512]
                if evac[nb] == "dve":
                    nc.vector.tensor_copy(dst, po[:, 0:512])
                else:
                    nc.scalar.copy(dst, po[:, 0:512])
                if split_out:
                    eng = nc.scalar if (m + nb) % 2 == 0 else nc.sync
                    eng.dma_start(out_d[ms, nb * 512:(nb + 1) * 512], dst)
            if not split_out:
                if out_q == "sp":
                    nc.sync.dma_start(out_d[ms, :], so[:])
                else:
                    nc.scalar.dma_start(out_d[ms, :], so[:])

        # ---------- interleaved emission ----------
        # Fine-grained round-robin: each score-group's exp (ACT) is shadowed
        # by a projection m-tile (PE) so the PE stream never blocks on the
        # single-buffered score PSUM tiles.
        pp = es.enter_context(tc.tile_pool(name="pp", bufs=1, space="PSUM"))
        for m in range(0, 4):
            proj_mtile(m)
        attention_scores_hp(0, 0, pe_main)
        proj_mtile(4)
        attention_scores_hp(0, 1, pe_main)
        proj_mtile(5)
        proj_mtile(6)
        proj_mtile(7)
        attention_pv(0, pe_main)
        attention_scores_hp(1, 0, pe_main)
        proj_mtile(8)
        attention_scores_hp(1, 1, pe_main)
        proj_mtile(9)
        proj_mtile(10)
        proj_mtile(11)
        attention_pv(1, pe_main)
        attention_scores_hp(2, 0, pe_main)
        proj_mtile(12)
        attention_scores_hp(2, 1, pe_main)
        proj_mtile(13)
        proj_mtile(14)
        proj_mtile(15)
        # phase 1 done: free xt/w/cos/sin, carve qc3 probs buffers from the
        # freed region so exp(qc3) is independent of PV(qc2)
        ph1_ctx.__exit__(None, None, None)
        with tc.tile_pool(name="pe3p", bufs=1) as pe3p:
            pe3 = [[pe3p.tile([128, NCOLS], BF16, tag=f"pe3{hp}{i}",
                              name=f"pe3{hp}{i}") for i in range(2)]
                   for hp in range(2)]
            attention_scores_hp(3, 0, pe3)
            for m in range(0, 4):
                oproj_m(m)
            attention_pv(2, pe_main)
            attention_scores_hp(3, 1, pe3)
            for m in range(4, 8):
                oproj_m(m)
            oproj_m(8, out_q="act")
            oproj_m(9, out_q="sp")
            oproj_m(10, out_q="act")
            oproj_m(11, out_q="sp")
            # tail: all four PV chains first (their normalize->transpose
            # chains pipeline down DVE/Pool/SP while PE works), then the
            # last o_proj tiles with po rotating through 4 banks and out
            # DMAs alternating between the SP and ACT queues
            attention_pv(3, pe3)
            oproj_m(12, tags=("ps_qk", "ps_v"), evac=("dve", "act"), out_q="act")
            oproj_m(13, tags=("sc0", "sc1"), evac=("dve", "act"), out_q="sp")
            oproj_m(14, tags=("ps_qk", "ps_v"), evac=("dve", "act"), split_out=True)
            oproj_m(15, tags=("yq", "sc0"), evac=("dve", "act"), split_out=True)
        es.close()
    nc.compile()
    return nc


_PERM64 = np.concatenate([np.arange(0, 64, 2), np.arange(1, 64, 2)])


def _prep_core_inputs(x, Wq, Wk, Wv, Wo, cos_g, sin_g, use_rope):
    """Host-side shard + layout prep. Returns list of 8 input dicts."""
    maskT = np.tril(np.ones((128, 128), np.float32)).T.astype(_BF16)
    # 16 copies of the 32-wide tables: [evens|odds] per head block x 8 blocks
    cos16 = np.tile(cos_g, (1, 16)).astype(_BF16)
    sin16 = np.tile(sin_g, (1, 16)).astype(_BF16)
    maps = []
    for c in range(NCORES):
        b, g = divmod(c, HEADS_PER_CORE)
        rows = slice(g * GDIM, (g + 1) * GDIM)
        wq_g = Wq[rows]
        wk_g = Wk[rows]
        if use_rope:
            # per-head row permutation to [evens(32) | odds(32)] so device
            # rope works on contiguous blocks; scores invariant (q,k share it)
            wq_g = wq_g.reshape(HEADS_PER_CORE, HD, D)[:, _PERM64, :].reshape(GDIM, D)
            wk_g = wk_g.reshape(HEADS_PER_CORE, HD, D)[:, _PERM64, :].reshape(GDIM, D)
        wqk = np.concatenate([wq_g, wk_g], axis=0).T  # [D, 512]
        maps.append({
            "xt": np.ascontiguousarray(x[b].T).astype(_BF16),
            "wqk": np.ascontiguousarray(wqk).astype(_BF16),
            "wv": np.ascontiguousarray(Wv[rows].T).astype(_BF16),
            "wo": np.ascontiguousarray(Wo[:, rows].T).astype(_BF16),
            "cos16": cos16,
            "sin16": sin16,
            "maskT": maskT,
        })
    return maps


def kernel(x, token_positions, use_rope, Wq, Wk, Wv, Wo, cos, sin):
    from concourse.bass_utils import run_bass_kernel_spmd

    x = np.asarray(x, np.float32)
    token_positions = np.asarray(token_positions)
    Wq = np.asarray(Wq, np.float32)
    Wk = np.asarray(Wk, np.float32)
    Wv = np.asarray(Wv, np.float32)
    Wo = np.asarray(Wo, np.float32)
    cos = np.asarray(cos, np.float32)
    sin = np.asarray(sin, np.float32)
    rope = bool(int(use_rope))

    cos_g = cos[token_positions]  # [S, 32]
    sin_g = sin[token_positions]

    if rope not in _cache:
        _cache[rope] = _build(rope)
    nc = _cache[rope]

    in_maps = _prep_core_inputs(x, Wq, Wk, Wv, Wo, cos_g, sin_g, rope)
    res = run_bass_kernel_spmd(nc, in_maps, list(range(NCORES)))

    out = np.zeros((B, S, D), np.float32)
    for c in range(NCORES):
        out[c // HEADS_PER_CORE] += res.results[c]["out"].astype(np.float32)
    return out


# revision 32
# speedup vs baseline: 1.2754x; 1.0007x over previous
"""Causal multi-head attention with RoPE for Trainium2, 8-core SPMD.

Problem: B=2, S=2048, D_MODEL=1024, H=16, HD=64, causal softmax(QK^T/8)V
with interleaved-pair RoPE on q/k, projections Wq/Wk/Wv/Wo.

Sharding (host side): batch x head-group. Core c handles batch b=c//4 and
head group g=c%4 (heads 4g..4g+3, a 256-wide slice of the projection dims).
Each core computes a full [S, D_MODEL] partial of the output (its head
group's contribution through Wo) in bf16; host casts to f32 and sums 4
partials per batch.

Device strategy (all matmuls bf16, fp32 accumulate):
 - emission interleaves projection m-tiles, attention q-chunks and o_proj
   tiles at score-group granularity so the PE stream always has work while
   ACT chews through the exp backlog; input DMAs are chunked so the first
   projection starts ~4.5us in
 - host permutes Wq/Wk rows per head to [evens(32) | odds(32)] so RoPE
   reads contiguous blocks: two full-width muls (qkf*cos16, qkf*sin16) +
   strided-block add/sub on DVE; scores are invariant to the permutation
   since q and k share it
 - Q,K projected in [s, o] layout -> RoPE -> one batched DMA transpose per
   m-tile into qkt4 [128, 4, S]; QK projection PSUM double-buffered across
   two tags, V accumulates via the PV psum ring
 - scoresT[k, q] = Kt.T @ Qt per 128-key block, head pairs row-packed on PE
   partitions 0:64/64:128; wide [128, 1024] PSUM score tiles, one Exp per
   (group, head) on ACT writing probs into per-(hp,head) SBUF buffers
   (qc0-2 share one set; qc3 gets its own carved from the released
   phase-1 pool so exp(qc3) never waits on PV(qc2)); causal diagonal
   masked by Pool multiply
 - PV flipped: out[q, h, hd] with lhsT = probs block [keys, q], rhs =
   [V | 1] [keys, 65] -- N=65 per matmul (the cost driver is the moving
   dim) instead of 128-512; col 64 accumulates the softmax denominator per
   q partition, so normalization is one reciprocal + per-partition
   tensor_scalar on Pool
 - y [q, hd] normalized then batch-transposed into yt2 [128, 2, S];
   o_proj per q-chunk with po PSUM rotating over freed phase-1/score
   banks, evacuation split DVE/ACT at the tail, out DMAs in bf16
   alternating between the SP and ACT DMA queues
"""

import numpy as np
import ml_dtypes

B, S, D, H = 2, 2048, 1024, 16
HD = 64
NCORES = 8
HEADS_PER_CORE = 4
GDIM = HEADS_PER_CORE * HD          # 256 projection cols per core
SB = S // 128                        # 16 s-tiles
KD = D // 128                        # 8 k-tiles over d
QCHUNK = 512
NQC = S // QCHUNK                    # 4 q-chunks
WIDE = 1024                          # wide scores psum tile (2 banks)

_BF16 = ml_dtypes.bfloat16
_cache = {}


def _score_layout(qc):
    """Per (qc): list of (kb, qoff, n) in emission order and the global column
    base of each kb block in the pe probs buffer; plus chunking into <=WIDE
    score-psum groups. Returns (groups, base) where groups is a list of
    [(kb, qoff, n, colbase), ...] and base maps kb -> global pe column."""
    q0 = qc * QCHUNK
    order = list(range(4 * qc)) + [4 * qc, 4 * qc + 1, 4 * qc + 3, 4 * qc + 2]
    base = {}
    blocks = []
    pos = 0
    for kb in order:
        r = max(0, kb - 4 * qc)
        qoff = q0 + r * 128 if kb >= 4 * qc else q0
        n = QCHUNK - r * 128 if kb >= 4 * qc else QCHUNK
        base[kb] = pos
        blocks.append((kb, qoff, n, pos))
        pos += n
    groups, cur, cols = [], [], 0
    for (kb, qoff, n, colbase) in blocks:
        if cols + n > WIDE:
            groups.append(cur)
            cur, cols = [], 0
        cur.append((kb, qoff, n, colbase))
        cols += n
    groups.append(cur)
    return groups, base, pos


def _build(use_rope: bool):
    import concourse.bass as bass
    import concourse.mybir as mybir
    import concourse.tile as tile
    from concourse import bacc
    from contextlib import ExitStack

    F32 = mybir.dt.float32
    BF16 = mybir.dt.bfloat16
    EXP = mybir.ActivationFunctionType.Exp
    MULT = mybir.AluOpType.mult

    nc = bacc.Bacc(None, target_bir_lowering=False)

    xt_d = nc.dram_tensor("xt", [D, S], BF16, kind="ExternalInput")
    wqk_d = nc.dram_tensor("wqk", [D, 2 * GDIM], BF16, kind="ExternalInput")
    wv_d = nc.dram_tensor("wv", [D, GDIM], BF16, kind="ExternalInput")
    wo_d = nc.dram_tensor("wo", [GDIM, D], BF16, kind="ExternalInput")
    cos_d = nc.dram_tensor("cos16", [S, 512], BF16, kind="ExternalInput")
    sin_d = nc.dram_tensor("sin16", [S, 512], BF16, kind="ExternalInput")
    mask_d = nc.dram_tensor("maskT", [128, 128], BF16, kind="ExternalInput")
    out_d = nc.dram_tensor("out", [S, D], BF16, kind="ExternalOutput")

    # pe probs buffer column count for the widest chunk (qc=3)
    _, _, NCOLS = _score_layout(NQC - 1)

    xt_dr = xt_d.rearrange("(k p) s -> p k s", p=128)
    wqk_dr = wqk_d.rearrange("(k p) o -> p k o", p=128)
    cos_dr = cos_d.rearrange("(m p) f -> p m f", p=128)
    sin_dr = sin_d.rearrange("(m p) f -> p m f", p=128)

    with tile.TileContext(nc) as tc:
        es = ExitStack()
        big = es.enter_context(tc.tile_pool(name="big", bufs=1))
        work = es.enter_context(tc.tile_pool(name="work", bufs=2))
        scp = es.enter_context(tc.tile_pool(name="sc", bufs=1, space="PSUM"))
        yqp = es.enter_context(tc.tile_pool(name="yq", bufs=2, space="PSUM"))

        # ---- resident tiles ----
        wo = big.tile([128, 2, D], BF16)
        maskT = big.tile([128, 128], BF16)
        qkt4 = big.tile([128, 4, S], BF16)
        vsb = big.tile([128, SB, HEADS_PER_CORE * 65], BF16)
        yt2 = big.tile([128, 2, S], BF16)
        # probs buffers for qc0-2 (max 5376 cols); qc3 gets its own buffers
        # carved out of the released phase-1 pool so exp(qc3) need not wait
        # for PV(qc2) to drain these
        _, _, NC2 = _score_layout(2)
        _, _, NC1 = _score_layout(1)
        pe_main = [[big.tile([128, NC2], BF16, tag=f"pe{hp}{i}",
                             name=f"pe{hp}{i}") for i in range(2)]
                   for hp in range(2)]
        # phase-1-only tensors: released after the last projection m-tile
        ph1_ctx = tc.tile_pool(name="ph1", bufs=1)
        ph1 = ph1_ctx.__enter__()
        xt = ph1.tile([128, KD, S], BF16)
        wqk = ph1.tile([128, KD, 2 * GDIM], BF16)
        wv = ph1.tile([128, KD, GDIM], BF16)
        if use_rope:
            cos16 = ph1.tile([128, SB, 512], BF16)
            sin16 = ph1.tile([128, SB, 512], BF16)

        # ones columns of [V | 1] (memset before anything else)
        vsb4 = vsb.rearrange("p m (h c) -> p m h c", h=4)
        nc.vector.memset(vsb4[:, :, :, 64:65], 1.0)

        # ---- input DMAs, chunked so m-tile 0 unblocks early; weights go
        # down the ACT queue in parallel with xt on the SP queue ----
        nc.sync.dma_start(wqk[:, 0:2, :], wqk_dr[:, 0:2, :])
        nc.sync.dma_start(xt[:, 0:2, 0:QCHUNK], xt_dr[:, 0:2, 0:QCHUNK])
        nc.sync.dma_start(wqk[:, 2:4, :], wqk_dr[:, 2:4, :])
        nc.sync.dma_start(xt[:, 2:4, 0:QCHUNK], xt_dr[:, 2:4, 0:QCHUNK])
        nc.sync.dma_start(wqk[:, 4:6, :], wqk_dr[:, 4:6, :])
        nc.sync.dma_start(xt[:, 4:6, 0:QCHUNK], xt_dr[:, 4:6, 0:QCHUNK])
        nc.sync.dma_start(wqk[:, 6:8, :], wqk_dr[:, 6:8, :])
        nc.sync.dma_start(xt[:, 6:8, 0:QCHUNK], xt_dr[:, 6:8, 0:QCHUNK])
        nc.sync.dma_start(wv[:], wv_d.rearrange("(k p) o -> p k o", p=128))
        if use_rope:
            nc.sync.dma_start(cos16[:, 0:4, :], cos_dr[:, 0:4, :])
            nc.sync.dma_start(sin16[:, 0:4, :], sin_dr[:, 0:4, :])
        nc.sync.dma_start(maskT[:], mask_d[:])
        for c in range(1, 4):
            cs = slice(c * QCHUNK, (c + 1) * QCHUNK)
            nc.sync.dma_start(xt[:, :, cs], xt_dr[:, :, cs])
            if use_rope:
                nc.sync.dma_start(cos16[:, 4*c:4*c+4, :], cos_dr[:, 4*c:4*c+4, :])
                nc.sync.dma_start(sin16[:, 4*c:4*c+4, :], sin_dr[:, 4*c:4*c+4, :])
        nc.sync.dma_start(wo[:], wo_d.rearrange("(k p) o -> p k o", p=128))

        # ---------- emission helpers ----------
        def proj_mtile(m):
            """QKV projection + rope + transpose + V staging for s-tile m."""
            ms = slice(m * 128, (m + 1) * 128)
            ps = pp.tile([128, 2 * GDIM], F32,
                         tag=("ps_qk", "ps_v")[m % 2], name="ps")
            for k in range(KD):
                nc.tensor.matmul(ps[:], xt[:, k, ms], wqk[:, k, :],
                                 start=(k == 0), stop=(k == KD - 1))
            qkr = work.tile([128, 2 * GDIM], BF16, tag="qkr", name="qkr")
            if use_rope:
                qkf = work.tile([128, 2 * GDIM], BF16, tag="qkf", name="qkf")
                if m < 4:
                    nc.scalar.copy(qkf[:], ps[:])
                else:
                    nc.vector.tensor_copy(qkf[:], ps[:])
                # head dims are [evens(32) | odds(32)] per 64-block (host
                # permuted).  Two full-width muls compute all four products:
                # t_c = qkf*cos = [E*c | O*c], t_s = qkf*sin = [E*s | O*s]
                ov = qkr.rearrange("p (hb eo f) -> p hb eo f", eo=2, f=32)
                t_c = work.tile([128, 512], BF16, tag="tc", name="tc")
                t_s = work.tile([128, 512], BF16, tag="ts", name="ts")
                nc.vector.tensor_mul(t_c[:], qkf[:], cos16[:, m, :])
                nc.vector.tensor_mul(t_s[:], qkf[:], sin16[:, m, :])
                tcv = t_c.rearrange("p (hb eo f) -> p hb eo f", eo=2, f=32)
                tsv = t_s.rearrange("p (hb eo f) -> p hb eo f", eo=2, f=32)
                # e' = E*c - O*s ; o' = O*c + E*s
                nc.vector.tensor_sub(ov[:, :, 0, :], tcv[:, :, 0, :], tsv[:, :, 1, :])
                nc.vector.tensor_add(ov[:, :, 1, :], tcv[:, :, 1, :], tsv[:, :, 0, :])
            else:
                nc.vector.tensor_copy(qkr[:], ps[:])
            # one batched transpose: [128 s, 512 o] -> qkt4[:, 0:4, m-block]
            gms = slice(m * 128, (m + 1) * 128)
            nc.sync.dma_start_transpose(qkt4[:, :, gms], qkr[:])

            psv = yqp.tile([128, GDIM], F32, tag="yq", name="psv")
            for k in range(KD):
                nc.tensor.matmul(psv[:], xt[:, k, ms], wv[:, k, :],
                                 start=(k == 0), stop=(k == KD - 1))
            dst = vsb4[:, m, :, 0:64]
            src = psv.rearrange("p (h c) -> p h c", h=4)
            if m < 4:
                nc.scalar.copy(dst, src)
            else:
                nc.vector.tensor_copy(dst, src)

        def attention_scores_hp(qc, hp, pe_all):
            """Scores + exp + causal mask for one head pair of q-chunk qc."""
            groups, base, ncols = _score_layout(qc)
            if True:
                qt = qkt4[:, hp, :]
                kt = qkt4[:, 2 + hp, :]
                for grp in groups:
                    gbase = grp[0][3]
                    gcols = grp[-1][3] + grp[-1][2] - gbase
                    scs = [scp.tile([128, WIDE], F32, tag=f"sc{i}",
                                    name=f"sc{i}") for i in range(2)]
                    for i in range(2):
                        rows = slice(i * 64, i * 64 + 64)
                        for (kb, qoff, n, colbase) in grp:
                            o = colbase - gbase
                            nc.tensor.matmul(
                                scs[i][:, o:o + n],
                                kt[rows, kb * 128:(kb + 1) * 128],
                                qt[rows, qoff:qoff + n],
                                start=True, stop=True)
                    for i in range(2):
                        pe = pe_all[hp][i]
                        nc.scalar.activation(pe[:, gbase:gbase + gcols],
                                             scs[i][:, :gcols], EXP, scale=0.125)
                        for (kb, qoff, n, colbase) in grp:
                            if kb >= 4 * qc:  # diagonal block: causal mask
                                nc.gpsimd.tensor_mul(
                                    pe[:, colbase:colbase + 128],
                                    pe[:, colbase:colbase + 128], maskT[:])

        def attention_pv(qc, pe_all, qls=(0, 1, 2, 3)):
            """Flipped PV per q-block: out [128 q, 4 heads, 65], then
            normalize via the accumulated denominator column + transpose."""
            _, base, _ = _score_layout(qc)
            for ql in qls:
                qb = 4 * qc + ql
                yq = yqp.tile([128, 4, 65], F32, tag="yq", name="yq")
                for h in range(4):
                    hp, i = divmod(h, 2)
                    pe = pe_all[hp][i]
                    for kb in range(qb + 1):
                        off = 128 * ql if kb < 4 * qc else 128 * (qb - kb)
                        col = base[kb] + off
                        nc.tensor.matmul(
                            yq[:, h, :], pe[:, col:col + 128],
                            vsb[:, kb, h * 65:(h + 1) * 65],
                            start=(kb == 0), stop=(kb == qb))
                yq_sb = work.tile([128, 4, 65], F32, tag="yqsb", name="yqsb")
                if qc == 0:
                    nc.scalar.copy(yq_sb[:], yq[:])
                else:
                    nc.vector.tensor_copy(yq_sb[:], yq[:])
                rc = work.tile([128, 4], F32, tag="rc", name="rc")
                nc.vector.reciprocal(rc[:], yq_sb[:, :, 64])
                y_sb = work.tile([128, 4, 64], BF16, tag="ysb", name="ysb")
                for h in range(4):
                    nc.gpsimd.tensor_scalar(y_sb[:, h, :], yq_sb[:, h, 0:64],
                                            rc[:, h:h + 1], None, MULT)
                nc.sync.dma_start_transpose(
                    yt2[:, :, qb * 128:(qb + 1) * 128], y_sb[:])

        def oproj_m(m, tags=("ps_qk", "ps_v"), evac=("dve", "dve"), out_q="sp",
                    split_out=False):
            # po reuses the phase-1 projection PSUM banks (tags rotate) --
            # avoids a pool boundary, which would order o_proj after every
            # phase-1 instruction.  After the last exp, the sc tags can join
            # the rotation for a deeper po pipeline.
            ms = slice(m * 128, (m + 1) * 128)
            so = work.tile([128, D], BF16, tag="so", name="so", bufs=4)
            for nb in range(2):
                if tags[nb] in ("ps_qk", "ps_v"):
                    po = pp.tile([128, 512], F32, tag=tags[nb], name="po")
                elif tags[nb] == "yq":
                    po = yqp.tile([128, 512], F32, tag="yq", name="po")
                else:
                    po = scp.tile([128, WIDE], F32, tag=tags[nb], name="po")
                for k in range(2):
                    nc.tensor.matmul(po[:, 0:512], yt2[:, k, ms],
                                     wo[:, k, nb * 512:(nb + 1) * 512],
                                     start=(k == 0), stop=(k == 1))
                dst = so[:, nb * 512:(nb + 1) * 512]
                if evac[nb] == "dve":
                    nc.vector.tensor_copy(dst, po[:, 0:512])
                else:
                    nc.scalar.copy(dst, po[:, 0:512])
                if split_out:
                    eng = nc.scalar if (m + nb) % 2 == 0 else nc.sync
                    eng.dma_start(out_d[ms, nb * 512:(nb + 1) * 512], dst)
            if not split_out:
                if out_q == "sp":
                    nc.sync.dma_start(out_d[ms, :], so[:])
                else:
                    nc.scalar.dma_start(out_d[ms, :], so[:])

        # ---------- interleaved emission ----------
        # Fine-grained round-robin: each score-group's exp (ACT) is shadowed
        # by a projection m-tile (PE) so the PE stream never blocks on the
        # single-buffered score PSUM tiles.
        pp = es.enter_context(tc.tile_pool(name="pp", bufs=1, space="PSUM"))
        for m in range(0, 4):
            proj_mtile(m)
        attention_scores_hp(0, 0, pe_main)
        proj_mtile(4)
        attention_scores_hp(0, 1, pe_main)
        proj_mtile(5)
        proj_mtile(6)
        proj_mtile(7)
        attention_pv(0, pe_main)
        attention_scores_hp(1, 0, pe_main)
        proj_mtile(8)
        attention_scores_hp(1, 1, pe_main)
        proj_mtile(9)
        proj_mtile(10)
        proj_mtile(11)
        attention_pv(1, pe_main)
        attention_scores_hp(2, 0, pe_main)
        proj_mtile(12)
        attention_scores_hp(2, 1, pe_main)
        proj_mtile(13)
        proj_mtile(14)
        proj_mtile(15)
        # phase 1 done: free xt/w/cos/sin, carve qc3 probs buffers from the
        # freed region so exp(qc3) is independent of PV(qc2)
        ph1_ctx.__exit__(None, None, None)
        with tc.tile_pool(name="pe3p", bufs=1) as pe3p:
            pe3 = [[pe3p.tile([128, NCOLS], BF16, tag=f"pe3{hp}{i}",
                              name=f"pe3{hp}{i}") for i in range(2)]
                   for hp in range(2)]
            attention_scores_hp(3, 0, pe3)
            for m in range(0, 4):
                oproj_m(m)
            attention_pv(2, pe_main)
            attention_scores_hp(3, 1, pe3)
            for m in range(4, 8):
                oproj_m(m)
            oproj_m(8, out_q="act")
            oproj_m(9, out_q="sp")
            oproj_m(10, out_q="act")
            oproj_m(11, out_q="sp")
            # tail: all four PV chains first (their normalize->transpose
            # chains pipeline down DVE/Pool/SP while PE works), then the
            # last o_proj tiles with po rotating through 4 banks and out
            # DMAs alternating between the SP and ACT queues
            attention_pv(3, pe3)
            oproj_m(12, tags=("ps_qk", "ps_v"), evac=("dve", "act"), out_q="act")
            oproj_m(13, tags=("sc0", "sc1"), evac=("dve", "act"), out_q="sp")
            oproj_m(14, tags=("ps_qk", "ps_v"), evac=("dve", "act"), split_out=True)
            oproj_m(15, tags=("yq", "sc0"), evac=("dve", "act"), split_out=True)
        es.close()
    nc.compile()
    return nc


_PERM64 = np.concatenate([np.arange(0, 64, 2), np.arange(1, 64, 2)])


def _prep_core_inputs(x, Wq, Wk, Wv, Wo, cos_g, sin_g, use_rope):
    """Host-side shard + layout prep. Returns list of 8 input dicts."""
    maskT = np.tril(np.ones((128, 128), np.float32)).T.astype(_BF16)
    # 16 copies of the 32-wide tables: [evens|odds] per head block x 8 blocks
    cos16 = np.tile(cos_g, (1, 16)).astype(_BF16)
    sin16 = np.tile(sin_g, (1, 16)).astype(_BF16)
    maps = []
    for c in range(NCORES):
        b, g = divmod(c, HEADS_PER_CORE)
        rows = slice(g * GDIM, (g + 1) * GDIM)
        wq_g = Wq[rows]
        wk_g = Wk[rows]
        if use_rope:
            # per-head row permutation to [evens(32) | odds(32)] so device
            # rope works on contiguous blocks; scores invariant (q,k share it)
            wq_g = wq_g.reshape(HEADS_PER_CORE, HD, D)[:, _PERM64, :].reshape(GDIM, D)
            wk_g = wk_g.reshape(HEADS_PER_CORE, HD, D)[:, _PERM64, :].reshape(GDIM, D)
        wqk = np.concatenate([wq_g, wk_g], axis=0).T  # [D, 512]
        m = {
            "xt": np.ascontiguousarray(x[b].T).astype(_BF16),
            "wqk": np.ascontiguousarray(wqk).astype(_BF16),
            "wv": np.ascontiguousarray(Wv[rows].T).astype(_BF16),
            "wo": np.ascontiguousarray(Wo[:, rows].T).astype(_BF16),
            "maskT": maskT,
        }
        if use_rope:
            m["cos16"] = cos16
            m["sin16"] = sin16
        maps.append(m)
    return maps


def kernel(x, token_positions, use_rope, Wq, Wk, Wv, Wo, cos, sin):
    from concourse.bass_utils import run_bass_kernel_spmd

    x = np.asarray(x, np.float32)
    token_positions = np.asarray(token_positions)
    Wq = np.asarray(Wq, np.float32)
    Wk = np.asarray(Wk, np.float32)
    Wv = np.asarray(Wv, np.float32)
    Wo = np.asarray(Wo, np.float32)
    cos = np.asarray(cos, np.float32)
    sin = np.asarray(sin, np.float32)
    rope = bool(int(use_rope))

    cos_g = cos[token_positions]  # [S, 32]
    sin_g = sin[token_positions]

    if rope not in _cache:
        _cache[rope] = _build(rope)
    nc = _cache[rope]

    in_maps = _prep_core_inputs(x, Wq, Wk, Wv, Wo, cos_g, sin_g, rope)
    res = run_bass_kernel_spmd(nc, in_maps, list(range(NCORES)))

    out = np.zeros((B, S, D), np.float32)
    for c in range(NCORES):
        out[c // HEADS_PER_CORE] += res.results[c]["out"].astype(np.float32)
    return out


# revision 42
# speedup vs baseline: 1.3056x; 1.0237x over previous
"""Causal multi-head attention with RoPE for Trainium2, 8-core SPMD.

Problem: B=2, S=2048, D_MODEL=1024, H=16, HD=64, causal softmax(QK^T/8)V
with interleaved-pair RoPE on q/k, projections Wq/Wk/Wv/Wo.

Sharding (host side): batch x head-group. Core c handles batch b=c//4 and
head group g=c%4 (heads 4g..4g+3, a 256-wide slice of the projection dims).
Each core computes a full [S, D_MODEL] partial of the output (its head
group's contribution through Wo) in bf16; host casts to f32 and sums 4
partials per batch.

Device strategy (all matmuls bf16, fp32 accumulate):
 - emission interleaves projection m-tiles, attention q-chunks and o_proj
   tiles at score-group granularity so the PE stream always has work while
   ACT chews through the exp backlog; input DMAs are chunked so the first
   projection starts ~4.5us in
 - host permutes Wq/Wk rows per head to [evens(32) | odds(32)] so RoPE
   reads contiguous blocks: two full-width muls (qkf*cos16, qkf*sin16) +
   strided-block add/sub on DVE; scores are invariant to the permutation
   since q and k share it
 - Q,K projected in [s, o] layout -> RoPE -> one batched DMA transpose per
   m-tile into qkt4 [128, 4, S]; QK projection PSUM double-buffered across
   two tags, V accumulates via the PV psum ring
 - scoresT[k, q] = Kt.T @ Qt per 128-key block, head pairs row-packed on PE
   partitions 0:64/64:128; wide [128, 1024] PSUM score tiles, one Exp per
   (group, head) on ACT writing probs into per-(hp,head) SBUF buffers
   (qc0-2 share one set; qc3 gets its own carved from the released
   phase-1 pool so exp(qc3) never waits on PV(qc2)); causal diagonal
   masked by Pool multiply
 - PV flipped: out[q, h, hd] with lhsT = probs block [keys, q], rhs =
   [V | 1] [keys, 65] -- N=65 per matmul (the cost driver is the moving
   dim) instead of 128-512; col 64 accumulates the softmax denominator per
   q partition, so normalization is one reciprocal + per-partition
   tensor_scalar on Pool
 - y [q, hd] normalized then batch-transposed into yt2 [128, 2, S];
   o_proj per q-chunk with po PSUM rotating over freed phase-1/score
   banks, evacuation split DVE/ACT at the tail, out DMAs in bf16
   alternating between the SP and ACT DMA queues
"""

import numpy as np
import ml_dtypes

B, S, D, H = 2, 2048, 1024, 16
HD = 64
NCORES = 8
HEADS_PER_CORE = 4
GDIM = HEADS_PER_CORE * HD          # 256 projection cols per core
SB = S // 128                        # 16 s-tiles
KD = D // 128                        # 8 k-tiles over d
QCHUNK = 512
NQC = S // QCHUNK                    # 4 q-chunks
WIDE = 1024                          # wide scores psum tile (2 banks)

_BF16 = ml_dtypes.bfloat16
_cache = {}


def _score_layout(qc):
    """Per (qc): list of (kb, qoff, n) in emission order and the global column
    base of each kb block in the pe probs buffer; plus chunking into <=WIDE
    score-psum groups. Returns (groups, base) where groups is a list of
    [(kb, qoff, n, colbase), ...] and base maps kb -> global pe column."""
    q0 = qc * QCHUNK
    order = list(range(4 * qc)) + [4 * qc, 4 * qc + 1, 4 * qc + 3, 4 * qc + 2]
    base = {}
    blocks = []
    pos = 0
    for kb in order:
        r = max(0, kb - 4 * qc)
        qoff = q0 + r * 128 if kb >= 4 * qc else q0
        n = QCHUNK - r * 128 if kb >= 4 * qc else QCHUNK
        base[kb] = pos
        blocks.append((kb, qoff, n, pos))
        pos += n
    groups, cur, cols = [], [], 0
    for (kb, qoff, n, colbase) in blocks:
        if cols + n > WIDE:
            groups.append(cur)
            cur, cols = [], 0
        cur.append((kb, qoff, n, colbase))
        cols += n
    groups.append(cur)
    return groups, base, pos


def _build(use_rope: bool):
    import concourse.bass as bass
    import concourse.mybir as mybir
    import concourse.tile as tile
    from concourse import bacc
    from contextlib import ExitStack

    F32 = mybir.dt.float32
    BF16 = mybir.dt.bfloat16
    EXP = mybir.ActivationFunctionType.Exp
    MULT = mybir.AluOpType.mult

    nc = bacc.Bacc(None, target_bir_lowering=False)

    xt_d = nc.dram_tensor("xt", [D, S], BF16, kind="ExternalInput")
    wqk_d = nc.dram_tensor("wqk", [D, 2 * GDIM], BF16, kind="ExternalInput")
    wv_d = nc.dram_tensor("wv", [D, GDIM], BF16, kind="ExternalInput")
    wo_d = nc.dram_tensor("wo", [GDIM, D], BF16, kind="ExternalInput")
    cos_d = nc.dram_tensor("cos8", [S, 256], BF16, kind="ExternalInput")
    sin_d = nc.dram_tensor("sin8", [S, 256], BF16, kind="ExternalInput")
    mask_d = nc.dram_tensor("maskT", [128, 128], BF16, kind="ExternalInput")
    out_d = nc.dram_tensor("out", [S, D], BF16, kind="ExternalOutput")

    # pe probs buffer column count for the widest chunk (qc=3)
    _, _, NCOLS = _score_layout(NQC - 1)

    xt_dr = xt_d.rearrange("(k p) s -> p k s", p=128)
    wqk_dr = wqk_d.rearrange("(k p) o -> p k o", p=128)
    cos_dr = cos_d.rearrange("(m p) f -> p m f", p=128)
    sin_dr = sin_d.rearrange("(m p) f -> p m f", p=128)

    with tile.TileContext(nc) as tc:
        es = ExitStack()
        big = es.enter_context(tc.tile_pool(name="big", bufs=1))
        work = es.enter_context(tc.tile_pool(name="work", bufs=2))
        scp = es.enter_context(tc.tile_pool(name="sc", bufs=1, space="PSUM"))
        yqp = es.enter_context(tc.tile_pool(name="yq", bufs=2, space="PSUM"))

        # ---- resident tiles ----
        wo = big.tile([128, 2, D], BF16)
        maskT = big.tile([128, 128], BF16)
        qkt4 = big.tile([128, 4, S], BF16)
        vsb = big.tile([128, SB, HEADS_PER_CORE * 65], BF16)
        yt2 = big.tile([128, 2, S], BF16)
        # probs buffers for qc0-2 (max 5376 cols); qc3 gets its own buffers
        # carved out of the released phase-1 pool so exp(qc3) need not wait
        # for PV(qc2) to drain these
        _, _, NC2 = _score_layout(2)
        _, _, NC1 = _score_layout(1)
        pe_main = [[big.tile([128, NC2], BF16, tag=f"pe{hp}{i}",
                             name=f"pe{hp}{i}") for i in range(2)]
                   for hp in range(2)]
        # phase-1-only tensors: released after the last projection m-tile
        ph1_ctx = tc.tile_pool(name="ph1", bufs=1)
        ph1 = ph1_ctx.__enter__()
        xt = ph1.tile([128, KD, S], BF16)
        wqk = ph1.tile([128, KD, 2 * GDIM], BF16)
        wv = ph1.tile([128, KD, GDIM], BF16)
        if use_rope:
            cos8 = ph1.tile([128, SB, 256], BF16)
            sin8 = ph1.tile([128, SB, 256], BF16)

        # ones columns of [V | 1] (memset before anything else)
        vsb4 = vsb.rearrange("p m (h c) -> p m h c", h=4)
        nc.vector.memset(vsb4[:, :, :, 64:65], 1.0)

        # ---- input DMAs, chunked so m-tile 0 unblocks early; weights go
        # down the ACT queue in parallel with xt on the SP queue ----
        nc.sync.dma_start(wqk[:, 0:2, :], wqk_dr[:, 0:2, :])
        nc.sync.dma_start(xt[:, 0:2, 0:QCHUNK], xt_dr[:, 0:2, 0:QCHUNK])
        nc.sync.dma_start(wqk[:, 2:4, :], wqk_dr[:, 2:4, :])
        nc.sync.dma_start(xt[:, 2:4, 0:QCHUNK], xt_dr[:, 2:4, 0:QCHUNK])
        nc.sync.dma_start(wqk[:, 4:6, :], wqk_dr[:, 4:6, :])
        nc.sync.dma_start(xt[:, 4:6, 0:QCHUNK], xt_dr[:, 4:6, 0:QCHUNK])
        nc.sync.dma_start(wqk[:, 6:8, :], wqk_dr[:, 6:8, :])
        nc.sync.dma_start(xt[:, 6:8, 0:QCHUNK], xt_dr[:, 6:8, 0:QCHUNK])
        nc.sync.dma_start(wv[:], wv_d.rearrange("(k p) o -> p k o", p=128))
        if use_rope:
            nc.sync.dma_start(cos8[:, 0:4, :], cos_dr[:, 0:4, :])
            nc.sync.dma_start(sin8[:, 0:4, :], sin_dr[:, 0:4, :])
        nc.sync.dma_start(maskT[:], mask_d[:])
        for c in range(1, 4):
            cs = slice(c * QCHUNK, (c + 1) * QCHUNK)
            nc.sync.dma_start(xt[:, :, cs], xt_dr[:, :, cs])
            if use_rope:
                nc.sync.dma_start(cos8[:, 4*c:4*c+4, :], cos_dr[:, 4*c:4*c+4, :])
                nc.sync.dma_start(sin8[:, 4*c:4*c+4, :], sin_dr[:, 4*c:4*c+4, :])
        nc.sync.dma_start(wo[:], wo_d.rearrange("(k p) o -> p k o", p=128))

        # ---------- emission helpers ----------
        def proj_mtile(m):
            """QKV projection + rope + transpose + V staging for s-tile m."""
            ms = slice(m * 128, (m + 1) * 128)
            ps = pp.tile([128, 2 * GDIM], F32,
                         tag=("ps_qk", "ps_v")[m % 2], name="ps")
            for k in range(KD):
                nc.tensor.matmul(ps[:], xt[:, k, ms], wqk[:, k, :],
                                 start=(k == 0), stop=(k == KD - 1))
            qkr = work.tile([128, 2 * GDIM], BF16, tag="qkr", name="qkr")
            if use_rope:
                qkf = work.tile([128, 2 * GDIM], BF16, tag="qkf", name="qkf")
                if m < 4:
                    nc.scalar.copy(qkf[:], ps[:])
                else:
                    nc.vector.tensor_copy(qkf[:], ps[:])
                # head dims are [evens(32) | odds(32)] per 64-block (host
                # permuted): E/O are 8 contiguous 32-col blocks at stride 64
                qv = qkf.rearrange("p (hb eo f) -> p hb eo f", eo=2, f=32)
                ov = qkr.rearrange("p (hb eo f) -> p hb eo f", eo=2, f=32)
                E, O = qv[:, :, 0, :], qv[:, :, 1, :]
                C = cos8[:, m, :].rearrange("p (hb f) -> p hb f", f=32)
                Sn = sin8[:, m, :].rearrange("p (hb f) -> p hb f", f=32)
                t_c = work.tile([128, 512], BF16, tag="tc", name="tc")
                t_s = work.tile([128, 512], BF16, tag="ts", name="ts")
                tcv = t_c.rearrange("p (hb eo f) -> p hb eo f", eo=2, f=32)
                tsv = t_s.rearrange("p (hb eo f) -> p hb eo f", eo=2, f=32)
                nc.vector.tensor_mul(tcv[:, :, 0, :], E, C)
                nc.vector.tensor_mul(tcv[:, :, 1, :], O, C)
                nc.vector.tensor_mul(tsv[:, :, 0, :], E, Sn)
                nc.vector.tensor_mul(tsv[:, :, 1, :], O, Sn)
                # e' = E*c - O*s ; o' = O*c + E*s
                nc.vector.tensor_sub(ov[:, :, 0, :], tcv[:, :, 0, :], tsv[:, :, 1, :])
                nc.vector.tensor_add(ov[:, :, 1, :], tcv[:, :, 1, :], tsv[:, :, 0, :])
            else:
                nc.vector.tensor_copy(qkr[:], ps[:])
            # one batched transpose: [128 s, 512 o] -> qkt4[:, 0:4, m-block]
            gms = slice(m * 128, (m + 1) * 128)
            nc.sync.dma_start_transpose(qkt4[:, :, gms], qkr[:])

            psv = yqp.tile([128, GDIM], F32, tag="yq", name="psv")
            for k in range(KD):
                nc.tensor.matmul(psv[:], xt[:, k, ms], wv[:, k, :],
                                 start=(k == 0), stop=(k == KD - 1))
            dst = vsb4[:, m, :, 0:64]
            src = psv.rearrange("p (h c) -> p h c", h=4)
            if m < 4:
                nc.scalar.copy(dst, src)
            else:
                nc.vector.tensor_copy(dst, src)

        def attention_scores_hp(qc, hp, pe_all):
            """Scores + exp + causal mask for one head pair of q-chunk qc."""
            groups, base, ncols = _score_layout(qc)
            if True:
                qt = qkt4[:, hp, :]
                kt = qkt4[:, 2 + hp, :]
                for grp in groups:
                    gbase = grp[0][3]
                    gcols = grp[-1][3] + grp[-1][2] - gbase
                    scs = [scp.tile([128, WIDE], F32, tag=f"sc{i}",
                                    name=f"sc{i}") for i in range(2)]
                    for i in range(2):
                        rows = slice(i * 64, i * 64 + 64)
                        for (kb, qoff, n, colbase) in grp:
                            o = colbase - gbase
                            nc.tensor.matmul(
                                scs[i][:, o:o + n],
                                kt[rows, kb * 128:(kb + 1) * 128],
                                qt[rows, qoff:qoff + n],
                                start=True, stop=True)
                    for i in range(2):
                        pe = pe_all[hp][i]
                        nc.scalar.activation(pe[:, gbase:gbase + gcols],
                                             scs[i][:, :gcols], EXP, scale=0.125)
                        for (kb, qoff, n, colbase) in grp:
                            if kb >= 4 * qc:  # diagonal block: causal mask
                                nc.gpsimd.tensor_mul(
                                    pe[:, colbase:colbase + 128],
                                    pe[:, colbase:colbase + 128], maskT[:])

        def attention_pv(qc, pe_all, qls=(0, 1, 2, 3)):
            """Flipped PV per q-block: out [128 q, 4 heads, 65], then
            normalize via the accumulated denominator column + transpose."""
            _, base, _ = _score_layout(qc)
            for ql in qls:
                qb = 4 * qc + ql
                yq = yqp.tile([128, 4, 65], F32, tag="yq", name="yq")
                for h in range(4):
                    hp, i = divmod(h, 2)
                    pe = pe_all[hp][i]
                    for kb in range(qb + 1):
                        off = 128 * ql if kb < 4 * qc else 128 * (qb - kb)
                        col = base[kb] + off
                        nc.tensor.matmul(
                            yq[:, h, :], pe[:, col:col + 128],
                            vsb[:, kb, h * 65:(h + 1) * 65],
                            start=(kb == 0), stop=(kb == qb))
                yq_sb = work.tile([128, 4, 65], F32, tag="yqsb", name="yqsb")
                if qc == 0:
                    nc.scalar.copy(yq_sb[:], yq[:])
                else:
                    nc.vector.tensor_copy(yq_sb[:], yq[:])
                rc = work.tile([128, 4], F32, tag="rc", name="rc")
                nc.vector.reciprocal(rc[:], yq_sb[:, :, 64])
                y_sb = work.tile([128, 4, 64], BF16, tag="ysb", name="ysb")
                for h in range(4):
                    nc.gpsimd.tensor_scalar(y_sb[:, h, :], yq_sb[:, h, 0:64],
                                            rc[:, h:h + 1], None, MULT)
                nc.sync.dma_start_transpose(
                    yt2[:, :, qb * 128:(qb + 1) * 128], y_sb[:])

        def oproj_m(m, tags=("ps_qk", "ps_v"), evac=("dve", "dve"), out_q="sp",
                    split_out=False):
            # po reuses the phase-1 projection PSUM banks (tags rotate) --
            # avoids a pool boundary, which would order o_proj after every
            # phase-1 instruction.  After the last exp, the sc tags can join
            # the rotation for a deeper po pipeline.
            ms = slice(m * 128, (m + 1) * 128)
            so = work.tile([128, D], BF16, tag="so", name="so", bufs=4)
            for nb in range(2):
                if tags[nb] in ("ps_qk", "ps_v"):
                    po = pp.tile([128, 512], F32, tag=tags[nb], name="po")
                elif tags[nb] == "yq":
                    po = yqp.tile([128, 512], F32, tag="yq", name="po")
                else:
                    po = scp.tile([128, WIDE], F32, tag=tags[nb], name="po")
                for k in range(2):
                    nc.tensor.matmul(po[:, 0:512], yt2[:, k, ms],
                                     wo[:, k, nb * 512:(nb + 1) * 512],
                                     start=(k == 0), stop=(k == 1))
                dst = so[:, nb * 512:(nb + 1) * 512]
                if evac[nb] == "dve":
                    nc.vector.tensor_copy(dst, po[:, 0:512])
                else:
                    nc.scalar.copy(dst, po[:, 0:512])
                if split_out:
                    eng = nc.scalar if (m + nb) % 2 == 0 else nc.sync
                    eng.dma_start(out_d[ms, nb * 512:(nb + 1) * 512], dst)
            if not split_out:
                if out_q == "sp":
                    nc.sync.dma_start(out_d[ms, :], so[:])
                else:
                    nc.scalar.dma_start(out_d[ms, :], so[:])

        # ---------- interleaved emission ----------
        # Fine-grained round-robin: each score-group's exp (ACT) is shadowed
        # by a projection m-tile (PE) so the PE stream never blocks on the
        # single-buffered score PSUM tiles.
        pp = es.enter_context(tc.tile_pool(name="pp", bufs=1, space="PSUM"))
        for m in range(0, 4):
            proj_mtile(m)
        attention_scores_hp(0, 0, pe_main)
        proj_mtile(4)
        attention_scores_hp(0, 1, pe_main)
        proj_mtile(5)
        proj_mtile(6)
        proj_mtile(7)
        attention_pv(0, pe_main)
        attention_scores_hp(1, 0, pe_main)
        proj_mtile(8)
        attention_scores_hp(1, 1, pe_main)
        proj_mtile(9)
        proj_mtile(10)
        proj_mtile(11)
        attention_pv(1, pe_main)
        attention_scores_hp(2, 0, pe_main)
        proj_mtile(12)
        attention_scores_hp(2, 1, pe_main)
        proj_mtile(13)
        proj_mtile(14)
        proj_mtile(15)
        # phase 1 done: free xt/w/cos/sin, carve qc3 probs buffers from the
        # freed region so exp(qc3) is independent of PV(qc2)
        ph1_ctx.__exit__(None, None, None)
        with tc.tile_pool(name="pe3p", bufs=1) as pe3p:
            pe3 = [[pe3p.tile([128, NCOLS], BF16, tag=f"pe3{hp}{i}",
                              name=f"pe3{hp}{i}") for i in range(2)]
                   for hp in range(2)]
            attention_scores_hp(3, 0, pe3)
            for m in range(0, 4):
                oproj_m(m)
            attention_pv(2, pe_main)
            attention_scores_hp(3, 1, pe3)
            for m in range(4, 8):
                oproj_m(m)
            oproj_m(8, out_q="act")
            oproj_m(9, out_q="sp")
            oproj_m(10, out_q="act")
            oproj_m(11, out_q="sp")
            # tail: all four PV chains first (their normalize->transpose
            # chains pipeline down DVE/Pool/SP while PE works), then the
            # last o_proj tiles with po rotating through 4 banks and out
            # DMAs alternating between the SP and ACT queues
            attention_pv(3, pe3)
            oproj_m(12, tags=("ps_qk", "ps_v"), evac=("dve", "act"), out_q="act")
            oproj_m(13, tags=("sc0", "sc1"), evac=("dve", "act"), out_q="sp")
            oproj_m(14, tags=("ps_qk", "ps_v"), evac=("dve", "act"), split_out=True)
            oproj_m(15, tags=("yq", "sc0"), evac=("dve", "act"), split_out=True)
        es.close()
    nc.compile()
    return nc


_PERM64 = np.concatenate([np.arange(0, 64, 2), np.arange(1, 64, 2)])


def _prep_core_inputs(x, Wq, Wk, Wv, Wo, cos_g, sin_g, use_rope):
    """Host-side shard + layout prep. Returns list of 8 input dicts."""
    maskT = np.tril(np.ones((128, 128), np.float32)).T.astype(_BF16)
    # 8 copies of the 32-wide tables: one per [evens|odds] head block
    cos8 = np.tile(cos_g, (1, 8)).astype(_BF16)
    sin8 = np.tile(sin_g, (1, 8)).astype(_BF16)
    maps = []
    for c in range(NCORES):
        b, g = divmod(c, HEADS_PER_CORE)
        rows = slice(g * GDIM, (g + 1) * GDIM)
        wq_g = Wq[rows]
        wk_g = Wk[rows]
        if use_rope:
            # per-head row permutation to [evens(32) | odds(32)] so device
            # rope works on contiguous blocks; scores invariant (q,k share it)
            wq_g = wq_g.reshape(HEADS_PER_CORE, HD, D)[:, _PERM64, :].reshape(GDIM, D)
            wk_g = wk_g.reshape(HEADS_PER_CORE, HD, D)[:, _PERM64, :].reshape(GDIM, D)
        wqk = np.concatenate([wq_g, wk_g], axis=0).T  # [D, 512]
        m = {
            "xt": np.ascontiguousarray(x[b].T).astype(_BF16),
            "wqk": np.ascontiguousarray(wqk).astype(_BF16),
            "wv": np.ascontiguousarray(Wv[rows].T).astype(_BF16),
            "wo": np.ascontiguousarray(Wo[:, rows].T).astype(_BF16),
            "maskT": maskT,
        }
        if use_rope:
            m["cos8"] = cos8
            m["sin8"] = sin8
        maps.append(m)
    return maps


def kernel(x, token_positions, use_rope, Wq, Wk, Wv, Wo, cos, sin):
    from concourse.bass_utils import run_bass_kernel_spmd

    x = np.asarray(x, np.float32)
    token_positions = np.asarray(token_positions)
    Wq = np.asarray(Wq, np.float32)
    Wk = np.asarray(Wk, np.float32)
    Wv = np.asarray(Wv, np.float32)
    Wo = np.asarray(Wo, np.float32)
    cos = np.asarray(cos, np.float32)
    sin = np.asarray(sin, np.float32)
    rope = bool(int(use_rope))

    cos_g = cos[token_positions]  # [S, 32]
    sin_g = sin[token_positions]

    if rope not in _cache:
        _cache[rope] = _build(rope)
    nc = _cache[rope]

    in_maps = _prep_core_inputs(x, Wq, Wk, Wv, Wo, cos_g, sin_g, rope)
    res = run_bass_kernel_spmd(nc, in_maps, list(range(NCORES)))

    out = np.zeros((B, S, D), np.float32)
    for c in range(NCORES):
        out[c // HEADS_PER_CORE] += res.results[c]["out"].astype(np.float32)
    return out
